# revision 1
# baseline (speedup 1.0000x reference)
"""TRN2 Bass kernel for nn_BlockMoVaE (attention + MoE/VE routing block).

Self-contained: accepts FULL inputs, shards across 8 NeuronCores, returns
FULL output.

Sharding:
  Phase 1 (attention + router logits): token-parallel. Core c handles the
    512-query strip [qoff, qoff+512) of batch b=c//4, qoff=512*(c%4).
    Activations are kept FEATURE-major ([feature, token]) so no on-device
    transposes are needed. K/V are computed for the whole batch on each
    core of the batch group, with key 128-tiles stored in a per-core
    ROTATED slot order (slot s holds absolute key tile (qoff/128+s)%16) so
    the causal boundary lands at static slots 0..3 in every core's
    (shared, SPMD) program; fully-masked future tiles are killed by a
    per-slot additive bias (-3e4) inside the exp activation.
  Phase 2 (expert-parallel sparse MoE): core e computes MLP expert e over
    only the tokens routed to it (host gathers columns, capacity-padded);
    VE (vocab-embedding expert) rows are host-gathered and weighted on
    device per token strip. Host does top-2 routing between launches and
    the final scatter-add/assembly.

Matmuls run as float32r (full PE rate, ~1e-4 rel err); PSUM accumulates
in fp32.
"""
import numpy as np

import concourse.bass as bass
import concourse.bacc as bacc
import concourse.mybir as mybir
import concourse.tile as tile
from concourse.bass_utils import run_bass_kernel_spmd

# ---- problem constants (hardcoded per contest rules) ----
B, T, C = 2, 2048, 1024
NH, NKV, HD = 16, 8, 64
E_MLP, E_VE, TOPK = 8, 2, 2
HID = 2048
VOCAB = 50257
EPS = 1e-6
NCORES = 8
S = 512            # tokens per core strip
NSLOT = T // 128   # 16 key tiles per batch
NG = 4             # kv column groups of 512
NCAP = 1024        # expert token capacity (phase 2)

f32 = mybir.dt.float32
f32r = mybir.dt.float32r
bf16 = mybir.dt.bfloat16
AF = mybir.ActivationFunctionType

_prog_cache = {}



def _register_consts(nc, values):
    for value in values:
        key = (f32, float(value))
        if key not in nc.const_aps.aps:
            t = nc.alloc_sbuf_tensor(f"constap-{value}", [128, 1], f32)
            nc.gpsimd.memset(t.ap(), float(value))
            nc.const_aps.aps[key] = t.ap()
    nc.all_engine_barrier()


# --------------------------------------------------------------------------
# Phase 1 builder: attention + residual + rmsnorm + router logits
# --------------------------------------------------------------------------
def build_phase1(window: int):
    nc = bacc.Bacc("TRN2", target_bir_lowering=False, debug=False,
                   num_devices=NCORES)

    xT_b = nc.dram_tensor("xT_b", [C, T], f32r, kind="ExternalInput").ap()
    xT_s = xT_b[:, 0:S]          # strip == rotated slots 0..3
    cosR_b = nc.dram_tensor("cosR_b", [128, T], f32, kind="ExternalInput").ap()
    sinR_b = nc.dram_tensor("sinR_b", [128, T], f32, kind="ExternalInput").ap()
    cosR_s = cosR_b[:, 0:S]
    sinR_s = sinR_b[:, 0:S]
    kbias = nc.dram_tensor("kbias", [128, NSLOT], f32, kind="ExternalInput").ap()
    wqT = nc.dram_tensor("wqT", [C, NH * HD], f32r, kind="ExternalInput").ap()
    wkT = nc.dram_tensor("wkT", [C, NKV * HD], f32r, kind="ExternalInput").ap()
    wvT = nc.dram_tensor("wvT", [C, NKV * HD], f32r, kind="ExternalInput").ap()
    woT = nc.dram_tensor("woT", [C, C], f32r, kind="ExternalInput").ap()
    rwT = nc.dram_tensor("rwT", [C, E_MLP + E_VE], f32, kind="ExternalInput").ap()

    x2_out = nc.dram_tensor("x2_out", [C, S], f32, kind="ExternalOutput").ap()
    xf_out = nc.dram_tensor("xf_out", [C, S], f32r, kind="ExternalOutput").ap()
    logit_out = nc.dram_tensor("logit_out", [E_MLP + E_VE, S], f32,
                               kind="ExternalOutput").ap()

    _register_consts(nc, [EPS])
    from contextlib import ExitStack
    with tile.TileContext(nc) as tc, ExitStack() as est:
        const = est.enter_context(tc.tile_pool(name="const", bufs=1))
        ropes = est.enter_context(tc.tile_pool(name="ropes", bufs=1))
        ropeb = est.enter_context(tc.tile_pool(name="ropeb", bufs=1))
        wstream = est.enter_context(tc.tile_pool(name="wstream", bufs=2))
        wvp = est.enter_context(tc.tile_pool(name="wvp", bufs=1))
        xin = est.enter_context(tc.tile_pool(name="xin", bufs=1))
        kvp = est.enter_context(tc.tile_pool(name="kv", bufs=1))
        qp = est.enter_context(tc.tile_pool(name="qp", bufs=1))
        work = est.enter_context(tc.tile_pool(name="work", bufs=2))
        rows = est.enter_context(tc.tile_pool(name="rows", bufs=1))
        pexp = est.enter_context(tc.tile_pool(name="pexp", bufs=3))
        ypool = est.enter_context(tc.tile_pool(name="ypool", bufs=1))
        x2p = est.enter_context(tc.tile_pool(name="x2p", bufs=1))
        ps_acc = est.enter_context(tc.tile_pool(name="ps_acc", bufs=2, space="PSUM"))
        ps_row = est.enter_context(tc.tile_pool(name="ps_row", bufs=1, space="PSUM"))
        ps_bc = est.enter_context(tc.tile_pool(name="ps_bc", bufs=1, space="PSUM"))
        ps_att = est.enter_context(tc.tile_pool(name="ps_att", bufs=2, space="PSUM"))

        # ---- constants ----
        ones_col_f = const.tile([128, 1], f32, name="ones_col_f")
        nc.vector.memset(ones_col_f[:], 1.0)
        ones_col = const.tile([128, 1], f32r, name="ones_col")
        nc.scalar.copy(ones_col[:], ones_col_f[:])
        ones_row_f = const.tile([1, 128], f32, name="ones_row_f")
        nc.vector.memset(ones_row_f[:], 1.0)
        ones_row = const.tile([1, 128], f32r, name="ones_row")
        nc.scalar.copy(ones_row[:], ones_row_f[:])
        onescols = const.tile([128, NKV, 1], f32, name="onescols")
        nc.vector.memset(onescols[:], 1.0)
        onescols_r = const.tile([128, NKV, 1], f32r, name="onescols_r")
        nc.vector.tensor_copy(onescols_r[:], onescols[:])
        kb = const.tile([128, NSLOT], f32, name="kb")
        nc.sync.dma_start(kb[:], kbias[:])

        cs = ropes.tile([128, S], f32, name="cs")
        nc.sync.dma_start(cs[:], cosR_s[:])
        ss = ropes.tile([128, S], f32, name="ss")
        nc.sync.dma_start(ss[:], sinR_s[:])

        rw_t = [const.tile([128, E_MLP + E_VE], f32, tag=f"rw{i}",
                           name=f"rw{i}") for i in range(8)]
        wv_t = [wvp.tile([128, NKV * HD], f32r, tag=f"wv{i}", name=f"wv{i}")
                for i in range(8)]
        for i in range(8):
            nc.sync.dma_start(rw_t[i][:], rwT[bass.ts(i, 128), :])
            nc.sync.dma_start(wv_t[i][:], wvT[bass.ts(i, 128), :])

        # ---- helper: rms broadcast for feature-major tiles ----
        def rms_stats(xtiles, n, nfeat):
            ssq = ps_row.tile([1, n], f32, tag="row", name="ssq")
            for i, xt in enumerate(xtiles):
                sq = work.tile([128, n], f32r, tag="sqstat", name="sqstat", bufs=1)
                nc.vector.tensor_mul(sq[:], xt[:], xt[:])
                nc.tensor.matmul(ssq[:], ones_col[:], sq[:],
                                 start=(i == 0), stop=(i == len(xtiles) - 1))
            srow = rows.tile([1, n], f32, tag="srow", name="srow")
            nc.scalar.activation(srow[:], ssq[:], AF.Sqrt,
                                 bias=EPS, scale=1.0 / nfeat)
            rrow = rows.tile([1, n], f32r, tag="rrow", name="rrow")
            with nc.allow_low_precision(reason="f32r rms bcast rows"):
                nc.vector.reciprocal(rrow[:], srow[:])
            bc = ps_bc.tile([128, n], f32, tag="bc", name="bc")
            nc.tensor.matmul(bc[:], ones_row[:], rrow[:], start=True, stop=True)
            bcs = work.tile([128, n], f32, tag="bcstat", name="bcstat", bufs=1)
            nc.scalar.copy(bcs[:], bc[:])
            return bcs

        # ---- helper: rope + per-head rmsnorm on a projection psum ----
        def rope_norm(ps, cos_ap, sin_ap, n, out_tile, col0):
            # ps: [128, 2n] pair psum: cols 0:n = projection, n:2n = the same
            # projection with 32-row blocks swapped (computed by a second
            # matmul group with a column-swapped lhsT AP)
            swp = work.tile([128, n], f32, tag="swp", name="swp")
            nc.vector.tensor_mul(swp[:], ps[:, n:2 * n], sin_ap)
            t1 = work.tile([128, n], f32, tag="ropet1", name="ropet1")
            nc.vector.tensor_mul(t1[:], ps[:, 0:n], cos_ap)
            nc.vector.tensor_add(swp[:], t1[:], swp[:])   # roped value
            sq = work.tile([128, n], f32r, tag="ropet1", name="ropesq")
            nc.vector.tensor_mul(sq[:], swp[:], swp[:])
            for hh in range(2):
                p0 = 64 * hh
                ssqh = ps_row.tile([1, n], f32, tag="row", name="ssqh")
                nc.tensor.matmul(ssqh[:], ones_col[p0:p0 + 64, :],
                                 sq[p0:p0 + 64, :], start=True, stop=True)
                srow = rows.tile([1, n], f32, tag="srow", name="hsrow")
                nc.scalar.activation(srow[:], ssqh[:], AF.Sqrt,
                                     bias=EPS, scale=1.0 / HD)
                rrow = rows.tile([1, n], f32r, tag="rrow", name="hrrow")
                with nc.allow_low_precision(reason="f32r rms bcast rows"):
                    nc.vector.reciprocal(rrow[:], srow[:])
                bch = ps_bc.tile([64, n], f32, tag="bc", name="bch")
                nc.tensor.matmul(bch[:], ones_row[:, :64], rrow[:],
                                 start=True, stop=True)
                bcs = work.tile([128, n], f32, tag="hbc", name="hbc")
                nc.scalar.copy(bcs[p0:p0 + 64, :], bch[:])
                nc.vector.tensor_mul(
                    out_tile[p0:p0 + 64, col0:col0 + n],
                    swp[p0:p0 + 64, :], bcs[p0:p0 + 64, :])

        # ================= strip pipeline (Q) =================
        xs_t = [xin.tile([128, S], f32r, tag=f"xi{i}", name=f"xs{i}")
                for i in range(8)]
        for i in range(8):
            nc.sync.dma_start(xs_t[i][:], xT_s[bass.ts(i, 128), :])
        bc_s = rms_stats([t[:].bitcast(f32) for t in xs_t], S, C)
        xn_s = []
        for i in range(8):
            xr = xs_t[i][:]
            nc.vector.tensor_mul(xr, xr.bitcast(f32), bc_s[:])  # in-place norm
            xn_s.append(xr)

        qT = [qp.tile([128, S], f32r, tag=f"qT{i}", name=f"qT{i}")
              for i in range(8)]
        for dq in range(8):
            q_ps = ps_acc.tile([128, 2 * S], f32, tag="acc", name="q_ps")
            wsl = wstream.tile([128, C], f32r, tag="wq", name="wq_sl", bufs=1)
            nc.sync.dma_start(
                wsl[:].rearrange("p (a m) -> p a m", m=128),
                wqT[:, bass.ts(dq, 128)].rearrange("(a p) m -> p a m", p=128))
            wsw = wstream.tile([128, C], f32r, tag="wqsw", name="wq_sw",
                               bufs=1)
            nc.scalar.copy(
                wsw[:],
                wsl[:].rearrange("p (a h q c) -> p a h q c",
                                 h=2, q=2, c=32)[:, :, :, ::-1, :])
            for ci in range(8):
                nc.tensor.matmul(q_ps[:, 0:S], wsl[:, bass.ts(ci, 128)],
                                 xn_s[ci], start=(ci == 0), stop=(ci == 7))
            for ci in range(8):
                nc.tensor.matmul(q_ps[:, S:2 * S], wsw[:, bass.ts(ci, 128)],
                                 xn_s[ci], start=(ci == 0), stop=(ci == 7))
            rope_norm(q_ps, cs[:], ss[:], S, qT[dq], 0)

        # ================= batch pipeline (K, V) =================
        kT = [kvp.tile([128, T], f32r, tag=f"kT{i}", name=f"kT{i}")
              for i in range(4)]
        vaug = [kvp.tile([128, NKV * (HD + 1)], f32r, tag=f"va{i}",
                         name=f"va{i}") for i in range(NSLOT)]
        for g in range(NG):
            xb_t = [xin.tile([128, S], f32r, tag=f"xi{i}", name=f"xb{i}")
                    for i in range(8)]
            for i in range(8):
                nc.sync.dma_start(xb_t[i][:], xT_b[bass.ts(i, 128),
                                                   bass.ts(g, S)])
            cbg = ropeb.tile([128, S], f32, tag="cbg", name="cbg")
            nc.sync.dma_start(cbg[:], cosR_b[:, bass.ts(g, S)])
            sbg = ropeb.tile([128, S], f32, tag="sbg", name="sbg")
            nc.sync.dma_start(sbg[:], sinR_b[:, bass.ts(g, S)])
            bc_b = rms_stats([t[:].bitcast(f32) for t in xb_t], S, C)
            xn_b = []
            for i in range(8):
                xr = xb_t[i][:]
                nc.vector.tensor_mul(xr, xr.bitcast(f32), bc_b[:])
                xn_b.append(xr)
            for dk in range(4):
                k_ps = ps_acc.tile([128, 2 * S], f32, tag="acc", name="k_ps")
                wsl = wstream.tile([128, C], f32r, tag="wk", name="wk_sl",
                                   bufs=2)
                nc.sync.dma_start(
                    wsl[:].rearrange("p (a m) -> p a m", m=128),
                    wkT[:, bass.ts(dk, 128)].rearrange("(a p) m -> p a m",
                                                       p=128))
                wsw = wstream.tile([128, C], f32r, tag="wksw",
                                   name="wk_sw", bufs=1)
                nc.scalar.copy(
                    wsw[:],
                    wsl[:].rearrange("p (a h q c) -> p a h q c",
                                     h=2, q=2, c=32)[:, :, :, ::-1, :])
                for ci in range(8):
                    nc.tensor.matmul(k_ps[:, 0:S], wsl[:, bass.ts(ci, 128)],
                                     xn_b[ci], start=(ci == 0), stop=(ci == 7))
                for ci in range(8):
                    nc.tensor.matmul(k_ps[:, S:2 * S],
                                     wsw[:, bass.ts(ci, 128)],
                                     xn_b[ci], start=(ci == 0), stop=(ci == 7))
                rope_norm(k_ps, cbg[:], sbg[:], S, kT[dk], g * S)
            for tt in range(4):
                slot = g * 4 + tt
                v_ps = ps_acc.tile([128, NKV * HD], f32, tag="acc", name="v_ps")
                for ci in range(8):
                    nc.tensor.matmul(v_ps[:],
                                     xn_b[ci][:, bass.ts(tt, 128)],
                                     wv_t[ci][:], start=(ci == 0), stop=(ci == 7))
                va = vaug[slot]
                va3 = va[:].rearrange("p (h d) -> p h d", d=HD + 1)
                vp3 = v_ps[:].rearrange("p (h d) -> p h d", d=HD)
                nc.scalar.copy(va3[:, :, 0:HD], vp3[:, :, :])
                nc.vector.tensor_copy(va3[:, :, HD:HD + 1], onescols_r[:])

        # ================= attention =================
        yT = [ypool.tile([128, S], f32r, tag=f"yT{i}", name=f"yT{i}")
              for i in range(8)]
        for h in range(NH):
            kh = h // 2                       # kv head
            dk, kp0 = kh // 2, 64 * (kh % 2)  # kT chunk/partition offset
            # q head layout is host-permuted so its partition base matches
            # the kv head base (matmul requires equal bases)
            dq, qp0 = 2 * (h // 4) + (h % 2), 64 * ((h // 2) % 2)
            assert qp0 == kp0
            yv = ps_att.tile([HD + 1, S], f32, tag="yv", name="yv", bufs=2)
            for sp in range(NSLOT // 2):
                # two slots share one 2-bank psum tile and one exp op; the
                # per-slot dead bias is pair-uniform (dead range is slots
                # 4..15-qoff/128, always whole pairs)
                s2 = ps_acc.tile([128, 2 * S], f32, tag="acc", name="s2")
                for half in range(2):
                    s = 2 * sp + half
                    nc.tensor.matmul(
                        s2[:, half * S:(half + 1) * S],
                        kT[dk][kp0:kp0 + 64, bass.ts(s, 128)],
                        qT[dq][qp0:qp0 + 64, :], start=True, stop=True)
                pT = pexp.tile([128, 2 * S], f32r, tag="pT", name="pT")
                nc.scalar.activation(pT[:], s2[:], AF.Exp,
                                     bias=kb[:, 2 * sp:2 * sp + 1], scale=0.125)
                for half in range(2):
                    s = 2 * sp + half
                    pTh = pT[:, half * S:(half + 1) * S]
                    if s < 4:
                        nc.gpsimd.affine_select(
                            pTh, pTh, pattern=[[1, S]], base=-128 * s,
                            channel_multiplier=-1,
                            compare_op=mybir.AluOpType.is_ge, fill=0.0)
                        if window < 512 - 128 * s:
                            nc.gpsimd.affine_select(
                                pTh, pTh, pattern=[[1, S]],
                                base=-128 * s - window, channel_multiplier=-1,
                                compare_op=mybir.AluOpType.is_le, fill=0.0)
                    else:
                        # cover partially AND fully window-cut past slots:
                        # a fully-cut slot may be pair-unmasked (kbias is
                        # pair-granular), so affine-zero it here
                        m = NSLOT - s
                        if window < 128 * m + 511:
                            nc.gpsimd.affine_select(
                                pTh, pTh, pattern=[[1, S]],
                                base=128 * m - window, channel_multiplier=-1,
                                compare_op=mybir.AluOpType.is_le, fill=0.0)
                    nc.tensor.matmul(yv[:], vaug[s][:, 65 * kh:65 * kh + 65],
                                     pTh, start=(s == 0), stop=(s == NSLOT - 1))
            ry = rows.tile([1, S], f32r, tag="ry", name="ry", bufs=1)
            with nc.allow_low_precision(reason="f32r softmax denom row"):
                nc.vector.reciprocal(ry[:], yv[HD:HD + 1, :])
            bc_y = ps_bc.tile([64, S], f32, tag="bc", name="bc_y")
            nc.tensor.matmul(bc_y[:], ones_row[:, :64], ry[:],
                             start=True, stop=True)
            bcy_s = work.tile([128, S], f32, tag="hbc", name="bcy")
            nc.vector.tensor_copy(bcy_s[qp0:qp0 + 64, :], bc_y[:])
            nc.vector.tensor_mul(yT[dq][qp0:qp0 + 64, :], yv[0:HD, :],
                                 bcy_s[qp0:qp0 + 64, :])

        # ================= wo + residual + xf + router =================
        x2w = []
        for co in range(8):
            # ps_row is idle during attention, so wo accumulation can
            # overlap the attention tail instead of queueing on "acc" slots
            at_ps = ps_row.tile([128, S], f32, tag="row", name="at_ps")
            wsl = wstream.tile([128, C], f32r, tag="wo", name="wo_sl", bufs=2)
            nc.sync.dma_start(
                wsl[:].rearrange("p (a m) -> p a m", m=128),
                woT[:, bass.ts(co, 128)].rearrange("(a p) m -> p a m", p=128))
            for ci in range(8):
                nc.tensor.matmul(at_ps[:], wsl[:, bass.ts(ci, 128)],
                                 yT[ci][:], start=(ci == 0), stop=(ci == 7))
            xs2 = xin.tile([128, S], f32r, tag=f"xi{co}", name=f"xs2_{co}")
            nc.sync.dma_start(xs2[:], xT_s[bass.ts(co, 128), :])
            x2 = x2p.tile([128, S], f32, tag="x2w", name="x2w")
            nc.vector.tensor_add(x2[:], at_ps[:], xs2[:].bitcast(f32))
            nc.sync.dma_start(x2_out[bass.ts(co, 128), :], x2[:])
            # xf stats accumulate inline while x2 is still in SBUF (avoids
            # waiting on the DRAM round trip for the stats pass)
            sqf = work.tile([128, S], f32r, tag="sqstat", name="sqf", bufs=1)
            nc.vector.tensor_mul(sqf[:], x2[:], x2[:])
            if co == 0:
                ssq_f = ps_bc.tile([1, S], f32, tag="bc", name="ssq_f")
            nc.tensor.matmul(ssq_f[:], ones_col[:], sqf[:],
                             start=(co == 0), stop=(co == 7))
            x2w.append(x2)
        srow_f = rows.tile([1, S], f32, tag="srow", name="srow_f")
        nc.scalar.activation(srow_f[:], ssq_f[:], AF.Sqrt,
                             bias=EPS, scale=1.0 / C)
        rrow_f = rows.tile([1, S], f32r, tag="rrow", name="rrow_f")
        with nc.allow_low_precision(reason="f32r rms bcast rows"):
            nc.vector.reciprocal(rrow_f[:], srow_f[:])
        bcps_f = ps_bc.tile([128, S], f32, tag="bc", name="bcps_f")
        nc.tensor.matmul(bcps_f[:], ones_row[:], rrow_f[:],
                         start=True, stop=True)
        bc_f = work.tile([128, S], f32, tag="bcstat", name="bc_f", bufs=1)
        nc.scalar.copy(bc_f[:], bcps_f[:])
        # re-read x2 (streamed) only for the normalize apply
        x2r = [xin.tile([128, S], f32, tag=f"xi{i}", name=f"x2r{i}")
               for i in range(8)]
        for i in range(8):
            nc.sync.dma_start(x2r[i][:], x2_out[bass.ts(i, 128), :])
        rt_ps = ps_row.tile([E_MLP + E_VE, S], f32, tag="row", name="rt_ps")
        for i in range(8):
            xf = x2p.tile([128, S], f32r, tag="xf", name="xf")
            nc.vector.tensor_mul(xf[:], x2r[i][:], bc_f[:])
            nc.sync.dma_start(xf_out[bass.ts(i, 128), :], xf[:])
            nc.tensor.matmul(rt_ps[:], rw_t[i][:], xf[:].bitcast(f32),
                             start=(i == 0), stop=(i == 7))
        lg = rows.tile([E_MLP + E_VE, S], f32, tag="lg", name="lg", bufs=1)
        nc.scalar.copy(lg[:], rt_ps[:])
        nc.sync.dma_start(logit_out[:], lg[:])

    nc.compile()
    return nc


# --------------------------------------------------------------------------
# Phase 2 builder: sparse expert MLP + VE weighting
# --------------------------------------------------------------------------
def build_phase2(ncap: int):
    nc = bacc.Bacc("TRN2", target_bir_lowering=False, debug=False,
                   num_devices=NCORES)
    NT = ncap // 256

    xfg = nc.dram_tensor("xfg", [C, ncap], f32r, kind="ExternalInput").ap()
    w_upT = nc.dram_tensor("w_upT", [C, HID], f32r, kind="ExternalInput").ap()
    w_downT = nc.dram_tensor("w_downT", [HID, C], f32r,
                             kind="ExternalInput").ap()
    gate = nc.dram_tensor("gate", [1, ncap], f32r, kind="ExternalInput").ap()
    ve0 = nc.dram_tensor("ve0", [S, C], f32, kind="ExternalInput").ap()
    ve1 = nc.dram_tensor("ve1", [S, C], f32, kind="ExternalInput").ap()
    ve_g = nc.dram_tensor("ve_g", [128, 8], f32, kind="ExternalInput").ap()

    moe_out = nc.dram_tensor("moe_out", [C, ncap], f32, kind="ExternalOutput").ap()
    ve_out = nc.dram_tensor("ve_out", [S, C], f32, kind="ExternalOutput").ap()

    from contextlib import ExitStack
    with tile.TileContext(nc) as tc, ExitStack() as est:
        const = est.enter_context(tc.tile_pool(name="const", bufs=1))
        wpool = est.enter_context(tc.tile_pool(name="wpool", bufs=1))
        hpool = est.enter_context(tc.tile_pool(name="hpool", bufs=1))
        stream = est.enter_context(tc.tile_pool(name="stream", bufs=2))
        work = est.enter_context(tc.tile_pool(name="work", bufs=2))
        ps_h = est.enter_context(tc.tile_pool(name="ps_h", bufs=3, space="PSUM"))
        ps_o = est.enter_context(tc.tile_pool(name="ps_o", bufs=3, space="PSUM"))
        ps_b = est.enter_context(tc.tile_pool(name="ps_b", bufs=2, space="PSUM"))

        ones_row_f = const.tile([1, 128], f32)
        nc.vector.memset(ones_row_f[:], 1.0)
        ones_row = const.tile([1, 128], f32r)
        nc.scalar.copy(ones_row[:], ones_row_f[:])

        up_t = [wpool.tile([128, HID], f32r, tag=f"up{i}", name=f"up{i}") for i in range(8)]
        dn_t = [wpool.tile([128, C], f32r, tag=f"dn{i}", name=f"dn{i}") for i in range(16)]
        for i in range(8):
            nc.sync.dma_start(up_t[i][:], w_upT[bass.ts(i, 128), :])
        for i in range(16):
            nc.sync.dma_start(dn_t[i][:], w_downT[bass.ts(i, 128), :])
        veg = const.tile([128, 8], f32)
        nc.sync.dma_start(veg[:], ve_g[:])
        gate_sb = const.tile([1, ncap], f32r)
        nc.sync.dma_start(gate_sb[:], gate[:])

        for nt in range(NT):
            csl = bass.ts(nt, 256)
            xf_t = [stream.tile([128, 256], f32r, tag=f"xf{i}", name=f"xf{i}")
                    for i in range(8)]
            for i in range(8):
                nc.sync.dma_start(xf_t[i][:], xfg[bass.ts(i, 128), csl])
            g_ps = ps_b.tile([128, 256], f32)
            nc.tensor.matmul(g_ps[:], ones_row[:], gate_sb[:, csl],
                             start=True, stop=True)
            g_bc = work.tile([128, 256], f32, tag="gbc", name="gbc")
            nc.scalar.copy(g_bc[:], g_ps[:])
            hT = [hpool.tile([128, 256], f32r, tag=f"hT{i}", name=f"hT{i}")
                  for i in range(16)]
            for hc in range(16):
                h_ps = ps_h.tile([128, 256], f32)
                for ci in range(8):
                    nc.tensor.matmul(h_ps[:], up_t[ci][:, bass.ts(hc, 128)],
                                     xf_t[ci][:], start=(ci == 0),
                                     stop=(ci == 7))
                hr = work.tile([128, 256], f32, tag="hrelu", name="hrelu")
                nc.scalar.activation(hr[:], h_ps[:], AF.Relu)
                nc.vector.tensor_mul(hT[hc][:], hr[:], hr[:])
            for co in range(8):
                o_ps = ps_o.tile([128, 256], f32)
                for hc in range(16):
                    nc.tensor.matmul(o_ps[:], dn_t[hc][:, bass.ts(co, 128)],
                                     hT[hc][:], start=(hc == 0),
                                     stop=(hc == 15))
                ot = work.tile([128, 256], f32, tag="ot", name="ot")
                nc.vector.tensor_mul(ot[:], o_ps[:], g_bc[:])
                nc.sync.dma_start(moe_out[bass.ts(co, 128), csl], ot[:])

        # VE weighting for own token strip (token-major)
        for tt in range(4):
            rsl = bass.ts(tt, 128)
            r0 = stream.tile([128, C], f32, tag="ver0", name="ver0")
            r1 = stream.tile([128, C], f32, tag="ver1", name="ver1")
            nc.sync.dma_start(r0[:], ve0[rsl, :])
            nc.sync.dma_start(r1[:], ve1[rsl, :])
            nc.vector.tensor_scalar_mul(r0[:], r0[:], veg[:, 2 * tt:2 * tt + 1])
            nc.vector.tensor_scalar_mul(r1[:], r1[:],
                                        veg[:, 2 * tt + 1:2 * tt + 2])
            nc.vector.tensor_add(r0[:], r0[:], r1[:])
            nc.sync.dma_start(ve_out[rsl, :], r0[:])

    nc.compile()
    return nc


# --------------------------------------------------------------------------
# Host orchestration
# --------------------------------------------------------------------------
def _phase1_inputs(x, cos, sin, window, wq, wk, wv, wo, router_w):
    """Build per-core in_maps for phase 1."""
    cosT = np.ascontiguousarray(cos[0, :, 0, :].T)  # (32, T)
    sinT = np.ascontiguousarray(sin[0, :, 0, :].T)
    cosR = np.tile(cosT, (4, 1)).astype(np.float32)          # (128, T)
    sinR = np.tile(np.vstack([sinT, -sinT]), (2, 1)).astype(np.float32)

    # q-head placement permutation (see attention loop): head h lives at
    # chunk 2*(h//4)+(h%2), partition base 64*((h//2)%2)
    colmap = np.zeros(NH * HD, np.int64)
    for h in range(NH):
        pos = (2 * (h // 4) + (h % 2)) * 128 + 64 * ((h // 2) % 2)
        colmap[pos:pos + HD] = np.arange(h * HD, (h + 1) * HD)
    wqT = np.ascontiguousarray(wq.T[:, colmap])
    wkT = np.ascontiguousarray(wk.T)
    wvT = np.ascontiguousarray(wv.T)
    woT = np.ascontiguousarray(wo.T[colmap, :])
    rwT = np.ascontiguousarray(router_w.T)

    in_maps = []
    perms = []
    for c in range(NCORES):
        b, qi = c // 4, c % 4
        qoff = S * qi
        q128 = qoff // 128
        perm = [(q128 + s) % NSLOT for s in range(NSLOT)]
        perms.append(perm)
        xT = x[b].T  # (C, T)
        xT_rot = np.ascontiguousarray(
            xT.reshape(C, NSLOT, 128)[:, perm, :].reshape(C, T))
        cosR_b = np.ascontiguousarray(
            cosR.reshape(128, NSLOT, 128)[:, perm, :].reshape(128, T))
        sinR_b = np.ascontiguousarray(
            sinR.reshape(128, NSLOT, 128)[:, perm, :].reshape(128, T))
        # per-slot alive bias
        kbias = np.zeros((128, NSLOT), np.float32)
        alive_s = np.zeros(NSLOT, bool)
        for s in range(NSLOT):
            kt = perm[s]
            # any (q in [qoff, qoff+511], k in [kt*128, kt*128+127]) with
            # k <= q and q - k <= window?
            dmin = qoff - (kt * 128 + 127)
            dmax = qoff + S - 1 - kt * 128
            alive_s[s] = (dmax >= 0) and (dmin <= window)
        for sp in range(NSLOT // 2):
            # the device applies one bias per slot PAIR; window-cut dead
            # slots in a live pair are zeroed by the device affine instead
            if not (alive_s[2 * sp] or alive_s[2 * sp + 1]):
                kbias[:, 2 * sp:2 * sp + 2] = -30000.0
        in_maps.append(dict(
            xT_b=xT_rot,
            cosR_b=cosR_b, sinR_b=sinR_b, kbias=kbias,
            wqT=wqT, wkT=wkT, wvT=wvT, woT=woT, rwT=rwT,
        ))
    return in_maps, perms


def _route(logits, router_bias):
    """Top-2 routing exactly as the reference (on host, f32)."""
    sig = (1.0 / (1.0 + np.exp(-logits.astype(np.float32)))).astype(np.float32)
    sel = sig + router_bias[None, :].astype(np.float32)
    idx = np.argsort(-sel, axis=1, kind="stable")[:, :TOPK]
    tw = np.take_along_axis(sig, idx, axis=1)
    tw = tw / tw.sum(axis=1, keepdims=True)
    N = logits.shape[0]
    sparse_w = np.zeros((N, E_MLP + E_VE), np.float32)
    np.put_along_axis(sparse_w, idx, tw, axis=1)
    return sparse_w


def kernel(**inputs):
    x = np.asarray(inputs["x"], np.float32)
    token_ids = np.asarray(inputs["token_ids"])
    cos = np.asarray(inputs["cos"], np.float32)
    sin = np.asarray(inputs["sin"], np.float32)
    window = int(np.asarray(inputs["window_size"]))
    wq, wk, wv, wo = (np.asarray(inputs[k], np.float32)
                      for k in ("wq", "wk", "wv", "wo"))
    w_up = np.asarray(inputs["w_up"], np.float32)
    w_down = np.asarray(inputs["w_down"], np.float32)
    router_w = np.asarray(inputs["router_w"], np.float32)
    router_bias = np.asarray(inputs["router_bias"], np.float32)
    ve_tables = np.asarray(inputs["ve_tables"], np.float32)

    key1 = ("p1", window)
    if key1 not in _prog_cache:
        _prog_cache[key1] = build_phase1(window)
    nc1 = _prog_cache[key1]

    in_maps, _ = _phase1_inputs(x, cos, sin, window, wq, wk, wv, wo, router_w)
    res1 = run_bass_kernel_spmd(nc1, in_maps, list(range(NCORES))).results

    x2T = np.concatenate([res1[c]["x2_out"] for c in range(NCORES)], axis=1)
    xfT = np.concatenate([res1[c]["xf_out"] for c in range(NCORES)], axis=1)
    logits = np.concatenate([res1[c]["logit_out"].T for c in range(NCORES)],
                            axis=0)  # (N, 10)

    N = B * T
    sparse_w = _route(logits, router_bias)

    # dispatch
    ncap = NCAP
    idx_list, n_list = [], []
    for e in range(E_MLP):
        idx_e = np.nonzero(sparse_w[:, e])[0]
        idx_list.append(idx_e)
        n_list.append(len(idx_e))
    max_n = max(n_list)
    while ncap < max_n:
        ncap *= 2

    key2 = ("p2", ncap)
    if key2 not in _prog_cache:
        _prog_cache[key2] = build_phase2(ncap)
    nc2 = _prog_cache[key2]

    tok = token_ids.reshape(-1)
    in_maps2 = []
    for c in range(NCORES):
        e = c
        idx_e = idx_list[e]
        xfg = np.zeros((C, ncap), np.float32)
        xfg[:, :n_list[e]] = xfT[:, idx_e]
        gate = np.zeros((1, ncap), np.float32)
        gate[0, :n_list[e]] = sparse_w[idx_e, e]
        s0 = c * S
        strip_tok = tok[s0:s0 + S]
        ve0 = np.ascontiguousarray(ve_tables[0][strip_tok])
        ve1 = np.ascontiguousarray(ve_tables[1][strip_tok])
        veg = np.zeros((128, 8), np.float32)
        for tt in range(4):
            for ee in range(E_VE):
                veg[:, 2 * tt + ee] = sparse_w[s0 + tt * 128:s0 + (tt + 1) * 128,
                                               E_MLP + ee]
        in_maps2.append(dict(
            xfg=xfg, w_upT=np.ascontiguousarray(w_up[e].T),
            w_downT=np.ascontiguousarray(w_down[e].T),
            gate=gate, ve0=ve0, ve1=ve1, ve_g=veg,
        ))
    res2 = run_bass_kernel_spmd(nc2, in_maps2, list(range(NCORES))).results

    out = np.ascontiguousarray(x2T.T)  # (N, C)
    for c in range(NCORES):
        out[c * S:(c + 1) * S] += res2[c]["ve_out"]
    for e in range(E_MLP):
        n_e = n_list[e]
        if n_e:
            out[idx_list[e]] += res2[e]["moe_out"][:, :n_e].T
    return out.reshape(B, T, C).astype(np.float32)



# revision 37
# speedup vs baseline: 1.6568x; 1.6568x over previous
"""TRN2 Bass kernel for nn_BlockMoVaE (attention + MoE/VE routing block).

Self-contained: accepts FULL inputs, shards across 8 NeuronCores, returns
FULL output.

Three SPMD launches with host re-layout between them (host does only data
movement / routing; all FLOPs stay on device).  The router top-2 decision
is discrete, so everything feeding the logits (all of phase 1) runs at
f32r precision; only the post-routing expert MLP uses bf16/fp8.

  L1a  (token-parallel, 512 tokens/core = 2 causally-balanced 256-strips):
       x-rms stats, raw-x f32r QKV projections, token-major rope +
       per-head rmsnorm (the per-token x-norm scalar cancels inside the
       head rmsnorm, so Q/K project raw x; V is scaled by r explicitly).
       Exports token-major q/k/v (f32).
  L1b  (token-parallel attention): host re-lays q/k/v into score-friendly
       f32r layouts.  Causal work is balanced by giving core ci the
       256-token strips {ci, 7-ci} of its batch; the static program
       computes (8, 16) key-128-slots for the two strips; fully-dead
       slots are killed by a rank-1 bias row (-30000) folded into the
       score matmul contraction, diagonal tiles by static affine_selects.
       Softmax denominators ride along as a ones-column of V.  Then
       wo + residual + xf rmsnorm + router logits.
  L2   (expert-parallel MoE): host routes top-2 and gathers tokens per
       expert with sqrt(gate) pre-scaling (relu^2 is 2-homogeneous so the
       gate factors exactly); bf16 up / fp8-DoubleRow down projections;
       VE rows host-gathered, gate-pre-scaled, summed on device.
"""
import numpy as np
import ml_dtypes

import concourse.bass as bass
import concourse.bacc as bacc
import concourse.mybir as mybir
import concourse.tile as tile
from concourse.bass_utils import run_bass_kernel_spmd
from concourse.alu_op_type import AluOpType
from contextlib import ExitStack
from collections import deque

# ---- problem constants (hardcoded per contest rules) ----
B, T, C = 2, 2048, 1024
NH, NKV, HD = 16, 8, 64
E_MLP, E_VE, TOPK = 8, 2, 2
HID = 2048
VOCAB = 50257
EPS = 1e-6
NCORES = 8
S = 512              # tokens per core
QT = 4               # 128-token tiles per core (2 strips of 256)
NSTRIP = 2
SW = 256             # strip width (queries)
NPOS = (8, 16)       # static key-slot count per strip
POS_BASE = (0, 8)    # slot base in kT layout (total 24)
NSLOT_TOT = 24
DEAD_BIAS = -30000.0
NCAP0 = 1024

f32 = mybir.dt.float32
f32r = mybir.dt.float32r
bf16 = mybir.dt.bfloat16
fp8e4 = mybir.dt.float8e4
AF = mybir.ActivationFunctionType
DR = mybir.MatmulPerfMode.DoubleRow
E4 = ml_dtypes.float8_e4m3
BF = ml_dtypes.bfloat16

_prog_cache = {}


def _register_consts(nc, values):
    for value in values:
        key = (f32, float(value))
        if key not in nc.const_aps.aps:
            t = nc.alloc_sbuf_tensor(f"constap-{value}", [128, 1], f32)
            nc.gpsimd.memset(t.ap(), float(value))
            nc.const_aps.aps[key] = t.ap()
    nc.all_engine_barrier()


# --------------------------------------------------------------------------
# L1a: x stats + QKV projection + rope + head-rms (token-major epilogues)
# --------------------------------------------------------------------------
def build_1a():
    nc = bacc.Bacc("TRN2", target_bir_lowering=False, debug=False,
                   num_devices=NCORES)

    x_fm = nc.dram_tensor("x_fm", [128, 8, S], f32r, kind="ExternalInput").ap()
    x_tm = nc.dram_tensor("x_tm", [128, QT, C], f32, kind="ExternalInput").ap()
    wq_t = nc.dram_tensor("wq_t", [128, 8, NH * HD], f32r,
                          kind="ExternalInput").ap()
    wk_t = nc.dram_tensor("wk_t", [128, 8, NKV * HD], f32r,
                          kind="ExternalInput").ap()
    wv_t = nc.dram_tensor("wv_t", [128, 8, NKV * HD], f32r,
                          kind="ExternalInput").ap()
    cos_tm = nc.dram_tensor("cos_tm", [128, QT, 32], f32,
                            kind="ExternalInput").ap()
    sin_tm = nc.dram_tensor("sin_tm", [128, QT, 32], f32,
                            kind="ExternalInput").ap()

    q_out = nc.dram_tensor("q_out", [128, QT, NH * HD], f32,
                           kind="ExternalOutput").ap()
    k_out = nc.dram_tensor("k_out", [128, QT, NKV * HD], f32,
                           kind="ExternalOutput").ap()
    v_out = nc.dram_tensor("v_out", [128, QT, NKV * HD], f32,
                           kind="ExternalOutput").ap()

    _register_consts(nc, [EPS])
    with tile.TileContext(nc) as tc, ExitStack() as est:
        wp = est.enter_context(tc.tile_pool(name="wp", bufs=1))
        work = est.enter_context(tc.tile_pool(name="work", bufs=2))
        outp = est.enter_context(tc.tile_pool(name="outp", bufs=1))
        ps_q = est.enter_context(tc.tile_pool(name="ps_q", bufs=2, space="PSUM"))
        ps_k = est.enter_context(tc.tile_pool(name="ps_k", bufs=2, space="PSUM"))
        ps_v = est.enter_context(tc.tile_pool(name="ps_v", bufs=2, space="PSUM"))

        xf = wp.tile([128, 8, S], f32r, name="xf")
        xt = wp.tile([128, QT, C], f32, name="xt")
        wq = wp.tile([128, 8, NH * HD], f32r, name="wq")
        wk = wp.tile([128, 8, NKV * HD], f32r, name="wk")
        wv = wp.tile([128, 8, NKV * HD], f32r, name="wv")
        cs = wp.tile([128, QT, 32], f32, name="cs")
        sn = wp.tile([128, QT, 32], f32, name="sn")
        for t_, d_ in ((xf, x_fm), (xt, x_tm), (wq, wq_t), (wk, wk_t),
                       (wv, wv_t), (cs, cos_tm), (sn, sin_tm)):
            nc.sync.dma_start(t_[:], d_[:])

        qe = outp.tile([128, QT, NH * HD], f32, name="qe")
        ke = outp.tile([128, QT, NKV * HD], f32, name="ke")
        ve = outp.tile([128, QT, NKV * HD], f32, name="ve")

        def rope_norm(ps, nh, t, out_tile):
            """Token-major rope + per-head rmsnorm from psum [128, nh*64]."""
            qs = work.tile([128, nh, HD], f32, tag=f"qs{nh}", name="qs")
            nc.scalar.copy(qs[:], ps[:].rearrange("p (h d) -> p h d", d=HD))
            cosb = cs[:, t:t + 1, :].broadcast_to([128, nh, 32])
            sinb = sn[:, t:t + 1, :].broadcast_to([128, nh, 32])
            rp = work.tile([128, nh, HD], f32, tag=f"rp{nh}", name="rp")
            a = work.tile([128, nh, 32], f32, tag=f"ra{nh}", name="ra")
            b = work.tile([128, nh, 32], f32, tag=f"rb{nh}", name="rb")
            c2 = work.tile([128, nh, 32], f32, tag=f"rc{nh}", name="rc")
            d2 = work.tile([128, nh, 32], f32, tag=f"rd{nh}", name="rd")
            nc.vector.tensor_mul(a[:], qs[:, :, 0:32], cosb)
            nc.vector.tensor_mul(b[:], qs[:, :, 32:64], sinb)
            nc.gpsimd.tensor_mul(c2[:], qs[:, :, 32:64], cosb)
            nc.gpsimd.tensor_mul(d2[:], qs[:, :, 0:32], sinb)
            nc.gpsimd.tensor_add(rp[:, :, 0:32], a[:], b[:])
            nc.vector.tensor_sub(rp[:, :, 32:64], c2[:], d2[:])
            sq = work.tile([128, nh, HD], f32, tag=f"sq{nh}", name="sq")
            nc.scalar.activation(sq[:], rp[:], AF.Square)
            hs = work.tile([128, nh, 1], f32, tag=f"hs{nh}", name="hs")
            nc.vector.tensor_reduce(out=hs[:], in_=sq[:], op=AluOpType.add,
                                    axis=mybir.AxisListType.X)
            sh = work.tile([128, nh, 1], f32, tag=f"sh{nh}", name="sh")
            nc.scalar.activation(sh[:], hs[:], AF.Sqrt, bias=EPS,
                                 scale=1.0 / HD)
            rh = work.tile([128, nh, 1], f32, tag=f"rh{nh}", name="rh")
            with nc.allow_low_precision(reason="head rms recip"):
                nc.vector.reciprocal(rh[:], sh[:])
            nc.vector.tensor_mul(
                out_tile[:].rearrange("p (h d) -> p h d", d=HD),
                rp[:], rh[:].broadcast_to([128, nh, HD]))

        for t in range(QT):
            # per-token inv-rms of x (V only; cancels inside Q/K head-rms)
            xsq = work.tile([128, C], f32, tag="xsq", name="xsq")
            nc.scalar.activation(xsq[:], xt[:, t, :], AF.Square)
            ssq = work.tile([128, 1], f32, tag="ssq", name="ssq")
            nc.vector.tensor_reduce(out=ssq[:], in_=xsq[:], op=AluOpType.add,
                                    axis=mybir.AxisListType.XYZW)
            sx = work.tile([128, 1], f32, tag="sx", name="sx")
            nc.scalar.activation(sx[:], ssq[:], AF.Sqrt, bias=EPS,
                                 scale=1.0 / C)
            r = work.tile([128, 1], f32, tag="r", name="r")
            with nc.allow_low_precision(reason="x rms recip"):
                nc.vector.reciprocal(r[:], sx[:])

            q_ps = ps_q.tile([128, NH * HD], f32, name="q_ps")
            k_ps = ps_k.tile([128, NKV * HD], f32, name="k_ps")
            v_ps = ps_v.tile([128, NKV * HD], f32, name="v_ps")
            for half in range(2):
                hsl = bass.ts(half, NH * HD // 2)
                for cc in range(8):
                    nc.tensor.matmul(q_ps[:, hsl],
                                     xf[:, cc, bass.ts(t, 128)],
                                     wq[:, cc, hsl],
                                     start=(cc == 0), stop=(cc == 7))
            for cc in range(8):
                nc.tensor.matmul(k_ps[:], xf[:, cc, bass.ts(t, 128)],
                                 wk[:, cc, :], start=(cc == 0), stop=(cc == 7))
            for cc in range(8):
                nc.tensor.matmul(v_ps[:], xf[:, cc, bass.ts(t, 128)],
                                 wv[:, cc, :], start=(cc == 0), stop=(cc == 7))

            rope_norm(q_ps, NH, t, qe[:, t, :])
            rope_norm(k_ps, NKV, t, ke[:, t, :])
            nc.vector.tensor_scalar_mul(ve[:, t, :], v_ps[:], r[:])

        nc.sync.dma_start(q_out[:], qe[:])
        nc.sync.dma_start(k_out[:], ke[:])
        nc.sync.dma_start(v_out[:], ve[:])

    nc.compile()
    return nc


# --------------------------------------------------------------------------
# L1b: attention + wo + residual + xf rmsnorm + router logits
# --------------------------------------------------------------------------
def build_1b(masked: bool):
    nc = bacc.Bacc("TRN2", target_bir_lowering=False, debug=False,
                   num_devices=NCORES)

    # q rows 0..63 = head dims, row 64 = 1.0 (rank-1 bias carrier)
    q_sc = nc.dram_tensor("q_sc", [65, NH, NSTRIP, SW], f32r,
                          kind="ExternalInput").ap()
    kt_sc = nc.dram_tensor("kt_sc", [65, NKV, NSLOT_TOT, 128], f32r,
                           kind="ExternalInput").ap()
    # v columns 0..63 = v dims, col 64 = 1.0 (softmax denominator)
    v_sc = nc.dram_tensor("v_sc", [128, NKV, NSLOT_TOT, 65], f32r,
                          kind="ExternalInput").ap()
    x_fm32 = nc.dram_tensor("x_fm32", [128, 8, S], f32,
                            kind="ExternalInput").ap()
    wo_sc = nc.dram_tensor("wo_sc", [64, 16, C], f32r,
                           kind="ExternalInput").ap()
    rw_sb = nc.dram_tensor("rw_sb", [128, 8, E_MLP + E_VE], f32,
                           kind="ExternalInput").ap()
    if masked:
        wmask = nc.dram_tensor("wmask", [128, NSLOT_TOT * SW], f32,
                               kind="ExternalInput").ap()

    x2_out = nc.dram_tensor("x2_out", [128, 8, S], f32,
                            kind="ExternalOutput").ap()
    xfb_out = nc.dram_tensor("xfb_out", [128, 8, S], bf16,
                             kind="ExternalOutput").ap()
    lg_out = nc.dram_tensor("lg_out", [E_MLP + E_VE, S], f32,
                            kind="ExternalOutput").ap()

    _register_consts(nc, [EPS])
    with tile.TileContext(nc) as tc, ExitStack() as est:
        wp = est.enter_context(tc.tile_pool(name="wp", bufs=1))
        ytp = est.enter_context(tc.tile_pool(name="ytp", bufs=1))

        yT = ytp.tile([64, NH, S], f32r, name="yT")

        with tc.tile_pool(name="ps_sc", bufs=2, space="PSUM") as ps_sc, \
             tc.tile_pool(name="ps_yv", bufs=1, space="PSUM") as ps_yv, \
             tc.tile_pool(name="ps_bc", bufs=1, space="PSUM") as ps_bc, \
             tc.tile_pool(name="attp", bufs=1) as attp, \
             tc.tile_pool(name="kvs", bufs=2) as kvs, \
             tc.tile_pool(name="ptp", bufs=9) as ptp, \
             tc.tile_pool(name="ivp", bufs=2) as ivp:
            q_t = attp.tile([65, NH, NSTRIP, SW], f32r, name="q_t")
            nc.sync.dma_start(q_t[:], q_sc[:])
            ones64f = attp.tile([1, 64], f32, name="ones64f")
            nc.vector.memset(ones64f[:], 1.0)
            ones64 = attp.tile([1, 64], f32r, name="ones64")
            nc.scalar.copy(ones64[:], ones64f[:])
            if masked:
                wm_t = attp.tile([128, NSLOT_TOT * SW], f32, name="wm_t")
                nc.sync.dma_start(wm_t[:], wmask[:])

            # stream kt/v per (strip, kv-pair); each slice loaded once
            kv_tiles = {}
            for strip in range(NSTRIP):
                n_s = NPOS[strip]
                for kp in range(4):
                    kt = kvs.tile([65, 2, n_s, 128], f32r,
                                  tag=f"kt{strip}", name=f"kt{strip}_{kp}")
                    vt = kvs.tile([128, 2, n_s, 65], f32r,
                                  tag=f"vt{strip}", name=f"vt{strip}_{kp}")
                    sl = slice(POS_BASE[strip], POS_BASE[strip] + n_s)
                    nc.sync.dma_start(kt[:], kt_sc[:, 2 * kp:2 * kp + 2, sl, :])
                    nc.sync.dma_start(vt[:], v_sc[:, 2 * kp:2 * kp + 2, sl, :])
                    kv_tiles[(strip, kp)] = (kt, vt)
            x_t = wp.tile([128, 8, S], f32, name="x_t")
            rw_t = wp.tile([128, 8, E_MLP + E_VE], f32, name="rw_t")
            nc.sync.dma_start(x_t[:], x_fm32[:])
            nc.sync.dma_start(rw_t[:], rw_sb[:])

            def emit_scores(strip, hg):
                """Scores + exp + mask for the 4 heads of kv-pair hg."""
                n_s = NPOS[strip]
                kt, _ = kv_tiles[(strip, hg)]
                chunks = [(c0, min(4, n_s - c0)) for c0 in range(0, n_s, 4)]
                pts = []
                for hi in range(4):
                    h = 4 * hg + hi
                    kvl = hi // 2          # kv head within the pair
                    pt_chunks = []
                    for c0, cn in chunks:
                        sc = ps_sc.tile([128, 4 * SW], f32, tag="sc",
                                        name="sc")
                        for s in range(cn):
                            nc.tensor.matmul(
                                sc[:, bass.ts(s, SW)],
                                kt[:, kvl, c0 + s, :],
                                q_t[:, h, strip, :],
                                start=True, stop=True)
                        pt = ptp.tile([128, 4 * SW], f32r, tag="pt",
                                      name=f"pt{strip}_{h}_{c0}")
                        nc.scalar.activation(pt[:, 0:cn * SW],
                                             sc[:, 0:cn * SW],
                                             AF.Exp, scale=0.125)
                        if masked:
                            base = (POS_BASE[strip] + c0) * SW
                            nc.vector.tensor_mul(
                                pt[:, 0:cn * SW], pt[:, 0:cn * SW],
                                wm_t[:, base:base + cn * SW])
                        else:
                            if c0 + cn == n_s:
                                # main diagonal tile (last slot): q-half 0
                                # is fully future -> zero; q-half 1: k <= q
                                off = (cn - 1) * SW
                                zsl = pt[:, off:off + 128]
                                nc.gpsimd.affine_select(
                                    zsl, zsl, pattern=[[1, 128]], base=-128,
                                    channel_multiplier=-1,
                                    compare_op=AluOpType.is_ge, fill=0.0)
                                dsl = pt[:, off + 128:off + 256]
                                nc.gpsimd.affine_select(
                                    dsl, dsl, pattern=[[1, 128]], base=0,
                                    channel_multiplier=-1,
                                    compare_op=AluOpType.is_ge, fill=0.0)
                                # sub-diagonal (slot n_s-2): q-half 0 k<=q
                                if cn >= 2:
                                    ssl = pt[:, off - SW:off - 128]
                                    nc.gpsimd.affine_select(
                                        ssl, ssl, pattern=[[1, 128]], base=0,
                                        channel_multiplier=-1,
                                        compare_op=AluOpType.is_ge, fill=0.0)
                        pt_chunks.append(pt)
                    pts.append(pt_chunks)
                return pts, chunks

            def emit_yv(strip, hg, pts, chunks):
                """p@v accumulate (ones col 64 -> den row 64) + normalize."""
                n_s = NPOS[strip]
                _, vt = kv_tiles[(strip, hg)]
                yv_ps = ps_yv.tile([65, 4 * SW], f32, tag="yv",
                                   name=f"yv{strip}_{hg}")
                for hi in range(4):
                    kvl = hi // 2
                    for (c0, cn), pt in zip(chunks, pts[hi]):
                        for s in range(cn):
                            nc.tensor.matmul(
                                yv_ps[:, bass.ts(hi, SW)],
                                vt[:, kvl, c0 + s, :],
                                pt[:, bass.ts(s, SW)],
                                start=(c0 + s == 0),
                                stop=(c0 + s == n_s - 1))
                iv = ivp.tile([1, 4 * SW], f32r, tag="iv", name="iv")
                with nc.allow_low_precision(reason="softmax recip"):
                    nc.vector.reciprocal(iv[:], yv_ps[64:65, :])
                bc_ps = ps_bc.tile([64, 4 * SW], f32, tag="bc", name="bc")
                for hi in range(4):
                    nc.tensor.matmul(bc_ps[:, bass.ts(hi, SW)],
                                     ones64[:], iv[0:1, bass.ts(hi, SW)],
                                     start=True, stop=True)
                bc_sb = ivp.tile([64, 4 * SW], f32, tag="bcs", name="bcs")
                nc.vector.tensor_copy(bc_sb[:], bc_ps[:])
                nc.vector.tensor_mul(
                    yT[:, 4 * hg:4 * hg + 4, bass.ts(strip, SW)],
                    yv_ps[0:64, :].rearrange("p (h n) -> p h n", h=4),
                    bc_sb[:].rearrange("p (h n) -> p h n", h=4))

            pending = deque()
            LAG = 2
            for strip in range(NSTRIP):
                for hg in range(4):
                    pts, chunks = emit_scores(strip, hg)
                    pending.append((strip, hg, pts, chunks))
                    if len(pending) > LAG:
                        emit_yv(*pending.popleft())
            while pending:
                emit_yv(*pending.popleft())

        # ---- wo + residual + xf rmsnorm + router ----
        with tc.tile_pool(name="ps_at", bufs=2, space="PSUM") as ps_at, \
             tc.tile_pool(name="ps_row", bufs=2, space="PSUM") as ps_row, \
             tc.tile_pool(name="ps_bcf", bufs=1, space="PSUM") as ps_bcf, \
             tc.tile_pool(name="tl", bufs=2) as tl, \
             tc.tile_pool(name="x2p", bufs=1) as x2p:
            ones_f = tl.tile([128, 1], f32, tag="onesf", name="ones_f", bufs=1)
            nc.vector.memset(ones_f[:], 1.0)
            ones_col = tl.tile([128, 1], f32r, tag="onesc", name="ones_col",
                               bufs=1)
            nc.scalar.copy(ones_col[:], ones_f[:])
            ones_rf = tl.tile([1, 128], f32, tag="onesrf", name="ones_rf",
                              bufs=1)
            nc.vector.memset(ones_rf[:], 1.0)
            ones_row = tl.tile([1, 128], f32r, tag="onesr", name="ones_row",
                               bufs=1)
            nc.scalar.copy(ones_row[:], ones_rf[:])

            x2w = x2p.tile([128, 8, S], f32, name="x2w")
            ssq_f = ps_bcf.tile([1, S], f32, tag="ssqf", name="ssq_f")
            rt_ps = ps_row.tile([E_MLP + E_VE, S], f32, tag="rt", name="rt_ps")
            for co in range(8):
                wo_t = tl.tile([64, 16, 128], f32r, tag="wo", name=f"wo{co}")
                nc.sync.dma_start(wo_t[:], wo_sc[:, :, bass.ts(co, 128)])
                at_ps = ps_at.tile([128, S], f32, tag="at", name="at_ps")
                for cc in range(16):
                    nc.tensor.matmul(
                        at_ps[:], wo_t[:, cc, :],
                        yT[:, cc, :],
                        start=(cc == 0), stop=(cc == 15))
                nc.vector.tensor_add(x2w[:, co, :], at_ps[:], x_t[:, co, :])
                nc.sync.dma_start(x2_out[:, co, :], x2w[:, co, :])
                sqf = tl.tile([128, S], f32r, tag="sqf", name="sqf")
                nc.scalar.activation(sqf[:], x2w[:, co, :], AF.Square)
                nc.tensor.matmul(ssq_f[:], ones_col[:], sqf[:],
                                 start=(co == 0), stop=(co == 7))
                nc.tensor.matmul(rt_ps[:], rw_t[:, co, :], x2w[:, co, :],
                                 start=(co == 0), stop=(co == 7))

            srow = tl.tile([1, S], f32, tag="srow", name="srow", bufs=1)
            nc.scalar.activation(srow[:], ssq_f[:], AF.Sqrt, bias=EPS,
                                 scale=1.0 / C)
            rrow = tl.tile([1, S], f32r, tag="rrow", name="rrow", bufs=1)
            with nc.allow_low_precision(reason="f32r rms bcast rows"):
                nc.vector.reciprocal(rrow[:], srow[:])
            bcf_ps = ps_row.tile([128, S], f32, tag="bcf", name="bcf_ps")
            nc.tensor.matmul(bcf_ps[:], ones_row[:], rrow[:],
                             start=True, stop=True)

            bcf_sb = tl.tile([128, S], f32, tag="bcfs", name="bcf_sb", bufs=1)
            nc.vector.tensor_copy(bcf_sb[:], bcf_ps[:])
            xfb = x2p.tile([128, 8, S], bf16, name="xfb")
            lg = tl.tile([E_MLP + E_VE, S], f32, tag="lg", name="lg", bufs=1)
            nc.vector.tensor_mul(lg[:], rt_ps[:], bcf_sb[0:E_MLP + E_VE, :])
            nc.sync.dma_start(lg_out[:], lg[:])
            for co in range(8):
                eng = nc.vector if co % 2 == 0 else nc.gpsimd
                eng.tensor_mul(xfb[:, co, :], x2w[:, co, :], bcf_sb[:])
                nc.sync.dma_start(xfb_out[:, co, :], xfb[:, co, :])

    nc.compile()
    return nc


# --------------------------------------------------------------------------
# L2: expert MLP (bf16 up, fp8-DR down, gate pre-folded) + VE sum
# --------------------------------------------------------------------------
def build_2(ncap: int):
    nc = bacc.Bacc("TRN2", target_bir_lowering=False, debug=False,
                   num_devices=NCORES)
    NT = ncap // 512

    xfg = nc.dram_tensor("xfg", [128, 8, ncap], bf16, kind="ExternalInput").ap()
    wup = nc.dram_tensor("wup", [128, 8, HID], bf16, kind="ExternalInput").ap()
    wdn = nc.dram_tensor("wdn", [128, 16, C], fp8e4, kind="ExternalInput").ap()
    ve0 = nc.dram_tensor("ve0", [128, QT, C], fp8e4, kind="ExternalInput").ap()
    ve1 = nc.dram_tensor("ve1", [128, QT, C], fp8e4, kind="ExternalInput").ap()

    moe_out = nc.dram_tensor("moe_out", [128, 8, ncap], bf16,
                             kind="ExternalOutput").ap()
    ve_out = nc.dram_tensor("ve_out", [128, QT, C], bf16,
                            kind="ExternalOutput").ap()

    with tile.TileContext(nc) as tc, ExitStack() as est:
        wp = est.enter_context(tc.tile_pool(name="wp", bufs=1))
        hp = est.enter_context(tc.tile_pool(name="hp", bufs=2))
        op = est.enter_context(tc.tile_pool(name="op", bufs=3))
        ps_h = est.enter_context(tc.tile_pool(name="ps_h", bufs=3, space="PSUM"))
        ps_o = est.enter_context(tc.tile_pool(name="ps_o", bufs=3, space="PSUM"))

        xf_t = wp.tile([128, 8, ncap], bf16, name="xf_t")
        up_t = wp.tile([128, 8, HID], bf16, name="up_t")
        dn_t = wp.tile([128, 16, C], fp8e4, name="dn_t")
        v0_t = wp.tile([128, QT, C], fp8e4, name="v0_t")
        v1_t = wp.tile([128, QT, C], fp8e4, name="v1_t")
        for t_, d_ in ((xf_t, xfg), (up_t, wup), (dn_t, wdn), (v0_t, ve0),
                       (v1_t, ve1)):
            nc.sync.dma_start(t_[:], d_[:])

        for nt in range(NT):
            csl = bass.ts(nt, 512)
            h_sb = hp.tile([128, 16, 512], fp8e4, tag="h", name=f"h{nt}")
            for hc in range(16):
                h_ps = ps_h.tile([128, 512], f32, tag="hps", name="h_ps")
                for cc in range(8):
                    nc.tensor.matmul(
                        h_ps[:], up_t[:, cc, bass.ts(hc, 128)],
                        xf_t[:, cc, csl],
                        start=(cc == 0), stop=(cc == 7))
                hr = op.tile([128, 512], bf16, tag="hr", name="hr")
                nc.scalar.activation(hr[:], h_ps[:], AF.Relu)
                eng = nc.vector if hc % 3 != 2 else nc.gpsimd
                eng.tensor_mul(h_sb[:, hc, :], hr[:], hr[:])
            for co in range(8):
                o_ps = ps_o.tile([128, 512], f32, tag="ops", name="o_ps")
                for hh in range(8):
                    nc.tensor.matmul(
                        o_ps[:], dn_t[:, 2 * hh:2 * hh + 2, bass.ts(co, 128)],
                        h_sb[:, 2 * hh:2 * hh + 2, :],
                        start=(hh == 0), stop=(hh == 7), perf_mode=DR)
                ot = op.tile([128, 512], bf16, tag="ot", name="ot")
                nc.scalar.copy(ot[:], o_ps[:])
                nc.sync.dma_start(moe_out[:, co, csl], ot[:])

        vo = op.tile([128, QT, C], bf16, tag="vo", name="vo", bufs=1)
        for t in range(QT):
            eng = nc.vector if t % 2 == 0 else nc.gpsimd
            eng.tensor_add(vo[:, t, :], v0_t[:, t, :], v1_t[:, t, :])
        nc.sync.dma_start(ve_out[:], vo[:])

    nc.compile()
    return nc


# --------------------------------------------------------------------------
# Host orchestration
# --------------------------------------------------------------------------
def _qtiles(ci):
    # strip A = 256-tile ci (128-tiles 2ci, 2ci+1), strip B = 256-tile 7-ci
    sa, sb = ci, 7 - ci
    return [2 * sa, 2 * sa + 1, 2 * sb, 2 * sb + 1]


def _slot_map(st, n_s, window):
    """slot -> (key 128-tile, bias).  Diagonal tile (2st+1) at slot n_s-1,
    sub-diagonal (2st) at n_s-2, other causally-alive tiles from slot 0."""
    alive = [kt for kt in range(2 * st)
             if window >= T or 128 * (2 * st + 1 - kt) - 127 <= window]
    m = {}
    for si, kt in enumerate(alive):
        m[si] = (kt, 0.0)
    m[n_s - 2] = (2 * st, 0.0)
    m[n_s - 1] = (2 * st + 1, 0.0)
    return m


def _route(logits, router_bias):
    sig = (1.0 / (1.0 + np.exp(-logits.astype(np.float32)))).astype(np.float32)
    sel = sig + router_bias[None, :].astype(np.float32)
    idx = np.argsort(-sel, axis=1, kind="stable")[:, :TOPK]
    tw = np.take_along_axis(sig, idx, axis=1)
    tw = tw / tw.sum(axis=1, keepdims=True)
    N = logits.shape[0]
    sparse_w = np.zeros((N, E_MLP + E_VE), np.float32)
    np.put_along_axis(sparse_w, idx, tw, axis=1)
    return sparse_w


def kernel(**inputs):
    x = np.asarray(inputs["x"], np.float32)
    token_ids = np.asarray(inputs["token_ids"])
    cos = np.asarray(inputs["cos"], np.float32)
    sin = np.asarray(inputs["sin"], np.float32)
    window = int(np.asarray(inputs["window_size"]))
    wq, wk, wv, wo = (np.asarray(inputs[k], np.float32)
                      for k in ("wq", "wk", "wv", "wo"))
    w_up = np.asarray(inputs["w_up"], np.float32)
    w_down = np.asarray(inputs["w_down"], np.float32)
    router_w = np.asarray(inputs["router_w"], np.float32)
    router_bias = np.asarray(inputs["router_bias"], np.float32)
    ve_tables = np.asarray(inputs["ve_tables"], np.float32)

    cosf = cos[0, :, 0, :]      # (T, 32)
    sinf = sin[0, :, 0, :]

    # ---------------- L1a ----------------
    if "1a" not in _prog_cache:
        _prog_cache["1a"] = build_1a()
    nc1a = _prog_cache["1a"]

    def mov_w(w):  # (M, C) -> [128, 8, M] with [p, cc, m] = w[m, 128cc+p]
        M = w.shape[0]
        return np.ascontiguousarray(
            w.T.reshape(8, 128, M).transpose(1, 0, 2)).astype(np.float32)

    wq_t, wk_t, wv_t = mov_w(wq), mov_w(wk), mov_w(wv)
    toks_all, maps1a = [], []
    for c in range(NCORES):
        b, ci = c // 4, c % 4
        qts = _qtiles(ci)
        toks = np.concatenate([np.arange(qt * 128, qt * 128 + 128)
                               for qt in qts])
        toks_all.append(toks)
        xs = x[b, toks, :]                      # (512, 1024)
        x_fm = np.ascontiguousarray(
            xs.T.reshape(8, 128, S).transpose(1, 0, 2)).astype(np.float32)
        x_tm = np.ascontiguousarray(
            xs.reshape(QT, 128, C).transpose(1, 0, 2)).astype(np.float32)
        cos_tm = np.ascontiguousarray(
            cosf[toks].reshape(QT, 128, 32).transpose(1, 0, 2)
        ).astype(np.float32)
        sin_tm = np.ascontiguousarray(
            sinf[toks].reshape(QT, 128, 32).transpose(1, 0, 2)
        ).astype(np.float32)
        maps1a.append(dict(x_fm=x_fm, x_tm=x_tm, wq_t=wq_t, wk_t=wk_t,
                           wv_t=wv_t, cos_tm=cos_tm, sin_tm=sin_tm))
    res1a = run_bass_kernel_spmd(nc1a, maps1a, list(range(NCORES))).results

    kn = np.zeros((B, T, NKV * HD), np.float32)
    vn = np.zeros((B, T, NKV * HD), np.float32)
    qn = []
    for c in range(NCORES):
        b = c // 4
        toks = toks_all[c]
        kc = res1a[c]["k_out"].astype(np.float32)   # [128, QT, 512]
        vc = res1a[c]["v_out"].astype(np.float32)
        kn[b, toks.reshape(QT, 128)] = kc.transpose(1, 0, 2)
        vn[b, toks.reshape(QT, 128)] = vc.transpose(1, 0, 2)
        qn.append(res1a[c]["q_out"].astype(np.float32))  # [128, QT, 1024]

    # ---------------- L1b ----------------
    masked = window < T
    key1b = ("1b", masked)
    if key1b not in _prog_cache:
        _prog_cache[key1b] = build_1b(masked)
    nc1b = _prog_cache[key1b]

    wo_sc = np.ascontiguousarray(
        wo.T.reshape(16, 64, C).transpose(1, 0, 2)).astype(np.float32)
    rw_sb = np.ascontiguousarray(
        router_w.T.reshape(8, 128, E_MLP + E_VE).transpose(1, 0, 2)
    ).astype(np.float32)

    maps1b = []
    for c in range(NCORES):
        b, ci = c // 4, c % 4
        strips = [ci, 7 - ci]          # 256-token strip indices
        toks = toks_all[c]
        q_sc = np.zeros((65, NH, NSTRIP, SW), np.float32)
        qc = qn[c]                     # [128, QT, 1024] token-major
        qtok = qc.transpose(1, 0, 2).reshape(S, NH, HD)   # (512, NH, 64)
        q_sc[0:64] = qtok.reshape(NSTRIP, SW, NH, HD).transpose(3, 2, 0, 1)
        q_sc[64] = 1.0
        kt_sc = np.zeros((65, NKV, NSLOT_TOT, 128), np.float32)
        kt_sc[64] = DEAD_BIAS
        v_sc = np.zeros((128, NKV, NSLOT_TOT, 65), np.float32)
        v_sc[:, :, :, 64] = 1.0
        kb = kn[b].reshape(16, 128, NKV, HD)   # [ktile, n, kv, d]
        vb = vn[b].reshape(16, 128, NKV, HD)
        for strip in range(NSTRIP):
            st = strips[strip]
            n_s = NPOS[strip]
            smap = _slot_map(st, n_s, window)
            for s, (kt, bias) in smap.items():
                po = POS_BASE[strip] + s
                kt_sc[0:64, :, po, :] = kb[kt].transpose(2, 1, 0)
                kt_sc[64, :, po, :] = bias
                v_sc[:, :, po, 0:64] = vb[kt]
        xs = x[b, toks, :]
        x_fm32 = np.ascontiguousarray(
            xs.T.reshape(8, 128, S).transpose(1, 0, 2)).astype(np.float32)
        m = dict(q_sc=q_sc, kt_sc=np.ascontiguousarray(kt_sc),
                 v_sc=np.ascontiguousarray(v_sc), x_fm32=x_fm32,
                 wo_sc=wo_sc, rw_sb=rw_sb)
        if masked:
            wm = np.zeros((128, NSLOT_TOT * SW), np.float32)
            for strip in range(NSTRIP):
                st = strips[strip]
                n_s = NPOS[strip]
                smap = _slot_map(st, n_s, window)
                for s, (kt, bias) in smap.items():
                    po = POS_BASE[strip] + s
                    qg = 2 * st * 128 + np.arange(SW)[None, :]
                    kg = kt * 128 + np.arange(128)[:, None]
                    ok = (kg <= qg) & (qg - kg <= window)
                    wm[:, po * SW:(po + 1) * SW] = ok
            m["wmask"] = wm.astype(np.float32)
        maps1b.append(m)
    res1b = run_bass_kernel_spmd(nc1b, maps1b, list(range(NCORES))).results

    # ---------------- routing ----------------
    N = B * T
    logits = np.zeros((N, E_MLP + E_VE), np.float32)
    x2 = np.zeros((N, C), np.float32)
    xfb = np.zeros((N, C), np.float32)
    for c in range(NCORES):
        b = c // 4
        toks = toks_all[c] + b * T
        logits[toks] = res1b[c]["lg_out"].T
        x2c = res1b[c]["x2_out"]                     # [128 p, 8 cc, 512 n]
        x2[toks] = x2c.transpose(2, 1, 0).reshape(S, C)
        xfc = res1b[c]["xfb_out"].astype(np.float32)
        xfb[toks] = xfc.transpose(2, 1, 0).reshape(S, C)

    sparse_w = _route(logits, router_bias)

    idx_list = [np.nonzero(sparse_w[:, e])[0] for e in range(E_MLP)]
    n_list = [len(ix) for ix in idx_list]
    ncap = NCAP0
    while ncap < max(n_list):
        ncap += 512

    key2 = ("2", ncap)
    if key2 not in _prog_cache:
        _prog_cache[key2] = build_2(ncap)
    nc2 = _prog_cache[key2]

    tok_flat = token_ids.reshape(-1)
    maps2 = []
    for c in range(NCORES):
        e = c
        ix = idx_list[e]
        g = np.sqrt(sparse_w[ix, e]).astype(np.float32)
        xg = np.zeros((C, ncap), np.float32)
        xg[:, :len(ix)] = xfb[ix].T * g[None, :]
        xfg = np.ascontiguousarray(
            xg.reshape(8, 128, ncap).transpose(1, 0, 2)).astype(BF)
        wup_m = np.ascontiguousarray(
            w_up[e].T.reshape(8, 128, HID).transpose(1, 0, 2)).astype(BF)
        wdn_dr = np.ascontiguousarray(
            w_down[e].T.reshape(16, 128, C).transpose(1, 0, 2)).astype(E4)
        b = c // 4
        toks = toks_all[c] + b * T
        tids = tok_flat[toks]
        ve0 = (64.0 * sparse_w[toks, E_MLP, None]
               * ve_tables[0][tids]).reshape(QT, 128, C).transpose(1, 0, 2)
        ve1 = (64.0 * sparse_w[toks, E_MLP + 1, None]
               * ve_tables[1][tids]).reshape(QT, 128, C).transpose(1, 0, 2)
        maps2.append(dict(xfg=xfg, wup=wup_m, wdn=wdn_dr,
                          ve0=np.ascontiguousarray(ve0).astype(E4),
                          ve1=np.ascontiguousarray(ve1).astype(E4)))
    res2 = run_bass_kernel_spmd(nc2, maps2, list(range(NCORES))).results

    out = x2.copy()
    for c in range(NCORES):
        b = c // 4
        toks = toks_all[c] + b * T
        veo = res2[c]["ve_out"].astype(np.float32)  # [128, QT, C]
        out[toks] += veo.transpose(1, 0, 2).reshape(S, C) / 64.0
    for e in range(E_MLP):
        n_e = n_list[e]
        if n_e:
            moe = res2[e]["moe_out"].astype(np.float32)  # [128, 8, ncap]
            out[idx_list[e]] += moe[:, :, :n_e].transpose(
                2, 1, 0).reshape(n_e, C)
    return out.reshape(B, T, C).astype(np.float32)


# revision 45
# speedup vs baseline: 1.9317x; 1.1660x over previous
"""TRN2 Bass kernel for nn_BlockMoVaE (attention + MoE/VE routing block).

Self-contained: accepts FULL inputs, shards across 8 NeuronCores, returns
FULL output.

Three SPMD launches with host re-layout between them (host does only data
movement / routing; all FLOPs stay on device).  The router top-2 decision
is discrete, so everything feeding the logits (all of phase 1) runs at
f32r precision; only the post-routing expert MLP uses bf16/fp8.

  L1a  (token-parallel, 512 tokens/core = 2 causally-balanced 256-strips):
       x-rms stats, raw-x f32r QKV projections, token-major rope +
       per-head rmsnorm (the per-token x-norm scalar cancels inside the
       head rmsnorm, so Q/K project raw x; V is scaled by r explicitly).
       Exports token-major q/k/v (f32).
  L1b  (token-parallel attention): host re-lays q/k/v into score-friendly
       f32r layouts.  Causal work is balanced by giving core ci the
       256-token strips {ci, 7-ci} of its batch; the static program
       computes (8, 16) key-128-slots for the two strips; fully-dead
       slots are killed by a rank-1 bias row (-30000) folded into the
       score matmul contraction, diagonal tiles by static affine_selects.
       Softmax denominators ride along as a ones-column of V.  Then
       wo + residual + xf rmsnorm + router logits.
  L2   (expert-parallel MoE): host routes top-2 and gathers tokens per
       expert with sqrt(gate) pre-scaling (relu^2 is 2-homogeneous so the
       gate factors exactly); bf16 up / fp8-DoubleRow down projections;
       VE rows host-gathered, gate-pre-scaled, summed on device.
"""
import numpy as np
import ml_dtypes

import concourse.bass as bass
import concourse.bacc as bacc
import concourse.mybir as mybir
import concourse.tile as tile
from concourse.bass_utils import run_bass_kernel_spmd
from concourse.alu_op_type import AluOpType
from contextlib import ExitStack
from collections import deque

# ---- problem constants (hardcoded per contest rules) ----
B, T, C = 2, 2048, 1024
NH, NKV, HD = 16, 8, 64
E_MLP, E_VE, TOPK = 8, 2, 2
HID = 2048
VOCAB = 50257
EPS = 1e-6
NCORES = 8
S = 512              # tokens per core
QT = 4               # 128-token tiles per core (2 strips of 256)
NSTRIP = 2
SW = 256             # strip width (queries)
NPOS = (8, 16)       # static key-slot count per strip
POS_BASE = (0, 8)    # slot base in kT layout (total 24)
NSLOT_TOT = 24
DEAD_BIAS = -30000.0
NCAP0 = 1024

f32 = mybir.dt.float32
f32r = mybir.dt.float32r
bf16 = mybir.dt.bfloat16
fp8e4 = mybir.dt.float8e4
AF = mybir.ActivationFunctionType
DR = mybir.MatmulPerfMode.DoubleRow
E4 = ml_dtypes.float8_e4m3
BF = ml_dtypes.bfloat16

_prog_cache = {}


def _register_consts(nc, values):
    for value in values:
        key = (f32, float(value))
        if key not in nc.const_aps.aps:
            t = nc.alloc_sbuf_tensor(f"constap-{value}", [128, 1], f32)
            nc.gpsimd.memset(t.ap(), float(value))
            nc.const_aps.aps[key] = t.ap()
    nc.all_engine_barrier()


# --------------------------------------------------------------------------
# L1a: x stats + QKV projection + rope + head-rms (token-major epilogues)
# --------------------------------------------------------------------------
def build_1a():
    nc = bacc.Bacc("TRN2", target_bir_lowering=False, debug=False,
                   num_devices=NCORES)

    x_fm = nc.dram_tensor("x_fm", [128, 8, S], f32r, kind="ExternalInput").ap()
    x_tm = nc.dram_tensor("x_tm", [128, QT, C], f32, kind="ExternalInput").ap()
    wq_t = nc.dram_tensor("wq_t", [128, 8, NH * HD], f32r,
                          kind="ExternalInput").ap()
    wk_t = nc.dram_tensor("wk_t", [128, 8, NKV * HD], f32r,
                          kind="ExternalInput").ap()
    wv_t = nc.dram_tensor("wv_t", [128, 8, NKV * HD], f32r,
                          kind="ExternalInput").ap()
    cos_tm = nc.dram_tensor("cos_tm", [128, QT, 32], f32,
                            kind="ExternalInput").ap()
    sin_tm = nc.dram_tensor("sin_tm", [128, QT, 32], f32,
                            kind="ExternalInput").ap()

    q_out = nc.dram_tensor("q_out", [128, QT, NH * HD], f32,
                           kind="ExternalOutput").ap()
    k_out = nc.dram_tensor("k_out", [128, QT, NKV * HD], f32,
                           kind="ExternalOutput").ap()
    v_out = nc.dram_tensor("v_out", [128, QT, NKV * HD], f32,
                           kind="ExternalOutput").ap()

    _register_consts(nc, [EPS])
    with tile.TileContext(nc) as tc, ExitStack() as est:
        wp = est.enter_context(tc.tile_pool(name="wp", bufs=1))
        work = est.enter_context(tc.tile_pool(name="work", bufs=2))
        outp = est.enter_context(tc.tile_pool(name="outp", bufs=1))
        ps_q = est.enter_context(tc.tile_pool(name="ps_q", bufs=2, space="PSUM"))
        ps_k = est.enter_context(tc.tile_pool(name="ps_k", bufs=2, space="PSUM"))
        ps_v = est.enter_context(tc.tile_pool(name="ps_v", bufs=2, space="PSUM"))

        xf = wp.tile([128, 8, S], f32r, name="xf")
        xt = wp.tile([128, QT, C], f32, name="xt")
        wq = wp.tile([128, 8, NH * HD], f32r, name="wq")
        wk = wp.tile([128, 8, NKV * HD], f32r, name="wk")
        wv = wp.tile([128, 8, NKV * HD], f32r, name="wv")
        cs = wp.tile([128, QT, 32], f32, name="cs")
        sn = wp.tile([128, QT, 32], f32, name="sn")
        nc.sync.dma_start(xf[:], x_fm[:])
        for cc in range(8):
            nc.sync.dma_start(wq[:, cc, :], wq_t[:, cc, :])
            nc.sync.dma_start(wk[:, cc, :], wk_t[:, cc, :])
            nc.sync.dma_start(wv[:, cc, :], wv_t[:, cc, :])
        nc.sync.dma_start(cs[:], cos_tm[:])
        nc.sync.dma_start(sn[:], sin_tm[:])
        nc.sync.dma_start(xt[:], x_tm[:])

        qe = outp.tile([128, QT, NH * HD], f32, name="qe")
        ke = outp.tile([128, QT, NKV * HD], f32, name="ke")
        ve = outp.tile([128, QT, NKV * HD], f32, name="ve")

        def rope_norm(ps, nh, t, out_tile):
            """Token-major rope + per-head rmsnorm from psum [128, nh*64]."""
            qs = work.tile([128, nh, HD], f32, tag=f"qs{nh}", name="qs")
            nc.scalar.copy(qs[:], ps[:].rearrange("p (h d) -> p h d", d=HD))
            cosb = cs[:, t:t + 1, :].broadcast_to([128, nh, 32])
            sinb = sn[:, t:t + 1, :].broadcast_to([128, nh, 32])
            rp = work.tile([128, nh, HD], f32, tag=f"rp{nh}", name="rp")
            a = work.tile([128, nh, 32], f32, tag=f"ra{nh}", name="ra")
            b = work.tile([128, nh, 32], f32, tag=f"rb{nh}", name="rb")
            c2 = work.tile([128, nh, 32], f32, tag=f"rc{nh}", name="rc")
            d2 = work.tile([128, nh, 32], f32, tag=f"rd{nh}", name="rd")
            nc.vector.tensor_mul(a[:], qs[:, :, 0:32], cosb)
            nc.vector.tensor_mul(b[:], qs[:, :, 32:64], sinb)
            nc.gpsimd.tensor_mul(c2[:], qs[:, :, 32:64], cosb)
            nc.gpsimd.tensor_mul(d2[:], qs[:, :, 0:32], sinb)
            nc.gpsimd.tensor_add(rp[:, :, 0:32], a[:], b[:])
            nc.vector.tensor_sub(rp[:, :, 32:64], c2[:], d2[:])
            sq = work.tile([128, nh, HD], f32, tag=f"sq{nh}", name="sq")
            nc.scalar.activation(sq[:], rp[:], AF.Square)
            hs = work.tile([128, nh, 1], f32, tag=f"hs{nh}", name="hs")
            nc.vector.tensor_reduce(out=hs[:], in_=sq[:], op=AluOpType.add,
                                    axis=mybir.AxisListType.X)
            sh = work.tile([128, nh, 1], f32, tag=f"sh{nh}", name="sh")
            nc.scalar.activation(sh[:], hs[:], AF.Sqrt, bias=EPS,
                                 scale=1.0 / HD)
            rh = work.tile([128, nh, 1], f32, tag=f"rh{nh}", name="rh")
            with nc.allow_low_precision(reason="head rms recip"):
                nc.vector.reciprocal(rh[:], sh[:])
            nc.vector.tensor_mul(
                out_tile[:].rearrange("p (h d) -> p h d", d=HD),
                rp[:], rh[:].broadcast_to([128, nh, HD]))

        for t in range(QT):
            # per-token inv-rms of x (V only; cancels inside Q/K head-rms)
            xsq = work.tile([128, C], f32, tag="xsq", name="xsq")
            nc.scalar.activation(xsq[:], xt[:, t, :], AF.Square)
            ssq = work.tile([128, 1], f32, tag="ssq", name="ssq")
            nc.vector.tensor_reduce(out=ssq[:], in_=xsq[:], op=AluOpType.add,
                                    axis=mybir.AxisListType.XYZW)
            sx = work.tile([128, 1], f32, tag="sx", name="sx")
            nc.scalar.activation(sx[:], ssq[:], AF.Sqrt, bias=EPS,
                                 scale=1.0 / C)
            r = work.tile([128, 1], f32, tag="r", name="r")
            with nc.allow_low_precision(reason="x rms recip"):
                nc.vector.reciprocal(r[:], sx[:])

            q_ps = ps_q.tile([128, NH * HD], f32, name="q_ps")
            k_ps = ps_k.tile([128, NKV * HD], f32, name="k_ps")
            v_ps = ps_v.tile([128, NKV * HD], f32, name="v_ps")
            for half in range(2):
                hsl = bass.ts(half, NH * HD // 2)
                for cc in range(8):
                    nc.tensor.matmul(q_ps[:, hsl],
                                     xf[:, cc, bass.ts(t, 128)],
                                     wq[:, cc, hsl],
                                     start=(cc == 0), stop=(cc == 7))
            for cc in range(8):
                nc.tensor.matmul(k_ps[:], xf[:, cc, bass.ts(t, 128)],
                                 wk[:, cc, :], start=(cc == 0), stop=(cc == 7))
            for cc in range(8):
                nc.tensor.matmul(v_ps[:], xf[:, cc, bass.ts(t, 128)],
                                 wv[:, cc, :], start=(cc == 0), stop=(cc == 7))

            rope_norm(q_ps, NH, t, qe[:, t, :])
            nc.sync.dma_start(q_out[:, t, :], qe[:, t, :])
            rope_norm(k_ps, NKV, t, ke[:, t, :])
            nc.sync.dma_start(k_out[:, t, :], ke[:, t, :])
            nc.vector.tensor_scalar_mul(ve[:, t, :], v_ps[:], r[:])
            nc.sync.dma_start(v_out[:, t, :], ve[:, t, :])

    nc.compile()
    return nc


# --------------------------------------------------------------------------
# L1b: attention + wo + residual + xf rmsnorm + router logits
# --------------------------------------------------------------------------
def build_1b(masked: bool):
    nc = bacc.Bacc("TRN2", target_bir_lowering=False, debug=False,
                   num_devices=NCORES)

    # q rows 0..63 = head dims, row 64 = 1.0 (rank-1 bias carrier)
    q_sc = nc.dram_tensor("q_sc", [65, NH, NSTRIP, SW], f32r,
                          kind="ExternalInput").ap()
    kt_sc = nc.dram_tensor("kt_sc", [65, NKV, NSLOT_TOT, 128], f32r,
                           kind="ExternalInput").ap()
    # v columns 0..63 = v dims, col 64 = 1.0 (softmax denominator)
    v_sc = nc.dram_tensor("v_sc", [128, NKV, NSLOT_TOT, 65], f32r,
                          kind="ExternalInput").ap()
    x_fm32 = nc.dram_tensor("x_fm32", [128, 8, S], f32,
                            kind="ExternalInput").ap()
    wo_sc = nc.dram_tensor("wo_sc", [128, 8, C], f32r,
                           kind="ExternalInput").ap()
    rw_sb = nc.dram_tensor("rw_sb", [128, 8, E_MLP + E_VE], f32,
                           kind="ExternalInput").ap()
    if masked:
        wmask = nc.dram_tensor("wmask", [128, NSLOT_TOT * SW], f32,
                               kind="ExternalInput").ap()

    x2_out = nc.dram_tensor("x2_out", [128, 8, S], f32,
                            kind="ExternalOutput").ap()
    xfb_out = nc.dram_tensor("xfb_out", [128, 8, S], f32,
                             kind="ExternalOutput").ap()
    lg_out = nc.dram_tensor("lg_out", [E_MLP + E_VE, S], f32,
                            kind="ExternalOutput").ap()

    _register_consts(nc, [EPS])
    with tile.TileContext(nc) as tc, ExitStack() as est:
        wp = est.enter_context(tc.tile_pool(name="wp", bufs=1))
        ytp = est.enter_context(tc.tile_pool(name="ytp", bufs=1))

        yTp = ytp.tile([128, NH // 2, S], f32r, name="yTp")
        yTo = ytp.tile([64, NH // 2, S], f32r, name="yTo")

        with tc.tile_pool(name="ps_sc", bufs=2, space="PSUM") as ps_sc, \
             tc.tile_pool(name="ps_yv", bufs=1, space="PSUM") as ps_yv, \
             tc.tile_pool(name="ps_bc", bufs=1, space="PSUM") as ps_bc, \
             tc.tile_pool(name="attp", bufs=1) as attp, \
             tc.tile_pool(name="kvs", bufs=2) as kvs, \
             tc.tile_pool(name="ptp", bufs=9) as ptp, \
             tc.tile_pool(name="ivp", bufs=2) as ivp:
            q_t = attp.tile([65, NH, NSTRIP, SW], f32r, name="q_t")
            for hg4 in range(4):
                nc.sync.dma_start(q_t[:, 4 * hg4:4 * hg4 + 4, 0, :],
                                  q_sc[:, 4 * hg4:4 * hg4 + 4, 0, :])
            for hg4 in range(4):
                nc.sync.dma_start(q_t[:, 4 * hg4:4 * hg4 + 4, 1, :],
                                  q_sc[:, 4 * hg4:4 * hg4 + 4, 1, :])
            ones64f = attp.tile([1, 64], f32, name="ones64f")
            nc.vector.memset(ones64f[:], 1.0)
            ones64 = attp.tile([1, 64], f32r, name="ones64")
            nc.scalar.copy(ones64[:], ones64f[:])
            if masked:
                wm_t = attp.tile([128, NSLOT_TOT * SW], f32, name="wm_t")
                nc.sync.dma_start(wm_t[:], wmask[:])

            # stream kt/v per (strip, kv-pair); each slice loaded once
            kv_tiles = {}
            for strip in range(NSTRIP):
                n_s = NPOS[strip]
                for kp in range(4):
                    kt = kvs.tile([65, 2, n_s, 128], f32r,
                                  tag=f"kt{strip}", name=f"kt{strip}_{kp}")
                    vt = kvs.tile([128, 2, n_s, 65], f32r,
                                  tag=f"vt{strip}", name=f"vt{strip}_{kp}")
                    sl = slice(POS_BASE[strip], POS_BASE[strip] + n_s)
                    nc.sync.dma_start(kt[:], kt_sc[:, 2 * kp:2 * kp + 2, sl, :])
                    nc.sync.dma_start(vt[:], v_sc[:, 2 * kp:2 * kp + 2, sl, :])
                    kv_tiles[(strip, kp)] = (kt, vt)
            x_t = wp.tile([128, 8, S], f32, name="x_t")
            rw_t = wp.tile([128, 8, E_MLP + E_VE], f32, name="rw_t")
            nc.sync.dma_start(x_t[:], x_fm32[:])
            nc.sync.dma_start(rw_t[:], rw_sb[:])

            def emit_scores(strip, hg):
                """Scores + exp + mask for the 4 heads of kv-pair hg."""
                n_s = NPOS[strip]
                kt, _ = kv_tiles[(strip, hg)]
                chunks = [(c0, min(4, n_s - c0)) for c0 in range(0, n_s, 4)]
                pts = []
                for hi in range(4):
                    h = 4 * hg + hi
                    kvl = hi // 2          # kv head within the pair
                    pt_chunks = []
                    for c0, cn in chunks:
                        sc = ps_sc.tile([128, 4 * SW], f32, tag="sc",
                                        name="sc")
                        for s in range(cn):
                            nc.tensor.matmul(
                                sc[:, bass.ts(s, SW)],
                                kt[:, kvl, c0 + s, :],
                                q_t[:, h, strip, :],
                                start=True, stop=True)
                        pt = ptp.tile([128, 4 * SW], f32r, tag="pt",
                                      name=f"pt{strip}_{h}_{c0}")
                        nc.scalar.activation(pt[:, 0:cn * SW],
                                             sc[:, 0:cn * SW],
                                             AF.Exp, scale=0.125)
                        if masked:
                            base = (POS_BASE[strip] + c0) * SW
                            nc.vector.tensor_mul(
                                pt[:, 0:cn * SW], pt[:, 0:cn * SW],
                                wm_t[:, base:base + cn * SW])
                        else:
                            if c0 + cn == n_s:
                                # main diagonal tile (last slot): q-half 0
                                # is fully future -> zero; q-half 1: k <= q
                                off = (cn - 1) * SW
                                zsl = pt[:, off:off + 128]
                                nc.gpsimd.affine_select(
                                    zsl, zsl, pattern=[[1, 128]], base=-128,
                                    channel_multiplier=-1,
                                    compare_op=AluOpType.is_ge, fill=0.0)
                                dsl = pt[:, off + 128:off + 256]
                                nc.gpsimd.affine_select(
                                    dsl, dsl, pattern=[[1, 128]], base=0,
                                    channel_multiplier=-1,
                                    compare_op=AluOpType.is_ge, fill=0.0)
                                # sub-diagonal (slot n_s-2): q-half 0 k<=q
                                if cn >= 2:
                                    ssl = pt[:, off - SW:off - 128]
                                    nc.gpsimd.affine_select(
                                        ssl, ssl, pattern=[[1, 128]], base=0,
                                        channel_multiplier=-1,
                                        compare_op=AluOpType.is_ge, fill=0.0)
                        pt_chunks.append(pt)
                    pts.append(pt_chunks)
                return pts, chunks

            def emit_yv(strip, hg, pts, chunks):
                """p@v accumulate (ones col 64 -> den row 64) + normalize."""
                n_s = NPOS[strip]
                _, vt = kv_tiles[(strip, hg)]
                yv_ps = ps_yv.tile([65, 4 * SW], f32, tag="yv",
                                   name=f"yv{strip}_{hg}")
                for hi in range(4):
                    kvl = hi // 2
                    for (c0, cn), pt in zip(chunks, pts[hi]):
                        for s in range(cn):
                            nc.tensor.matmul(
                                yv_ps[:, bass.ts(hi, SW)],
                                vt[:, kvl, c0 + s, :],
                                pt[:, bass.ts(s, SW)],
                                start=(c0 + s == 0),
                                stop=(c0 + s == n_s - 1))
                iv = ivp.tile([1, 4 * SW], f32r, tag="iv", name="iv")
                with nc.allow_low_precision(reason="softmax recip"):
                    nc.vector.reciprocal(iv[:], yv_ps[64:65, :])
                bc_ps = ps_bc.tile([64, 4 * SW], f32, tag="bc", name="bc")
                for hi in range(4):
                    nc.tensor.matmul(bc_ps[:, bass.ts(hi, SW)],
                                     ones64[:], iv[0:1, bass.ts(hi, SW)],
                                     start=True, stop=True)
                bc_sb = ivp.tile([64, 4 * SW], f32, tag="bcs", name="bcs")
                nc.vector.tensor_copy(bc_sb[:], bc_ps[:])
                yv4 = yv_ps[0:64, :].rearrange("p (h n) -> p h n", h=4)
                bc4 = bc_sb[:].rearrange("p (h n) -> p h n", h=4)
                ssl = bass.ts(strip, SW)
                # even heads (hi 0,2) -> chunks 2hg..2hg+1 rows 0:64
                nc.vector.tensor_mul(
                    yTp[0:64, 2 * hg:2 * hg + 2, ssl],
                    yv4[:, 0:4:2, :], bc4[:, 0:4:2, :])
                # odd heads -> staging, then partition-shift DMA
                nc.vector.tensor_mul(
                    yTo[:, 2 * hg:2 * hg + 2, ssl],
                    yv4[:, 1:4:2, :], bc4[:, 1:4:2, :])
                nc.sync.dma_start(yTp[64:128, 2 * hg:2 * hg + 2, ssl],
                                  yTo[:, 2 * hg:2 * hg + 2, ssl])

            pending = deque()
            LAG = 2
            for strip in range(NSTRIP):
                for hg in range(4):
                    pts, chunks = emit_scores(strip, hg)
                    pending.append((strip, hg, pts, chunks))
                    if len(pending) > LAG:
                        emit_yv(*pending.popleft())
            while pending:
                emit_yv(*pending.popleft())

        # ---- wo + residual + xf rmsnorm + router ----
        with tc.tile_pool(name="ps_at", bufs=2, space="PSUM") as ps_at, \
             tc.tile_pool(name="ps_row", bufs=2, space="PSUM") as ps_row, \
             tc.tile_pool(name="ps_bcf", bufs=1, space="PSUM") as ps_bcf, \
             tc.tile_pool(name="tl", bufs=2) as tl, \
             tc.tile_pool(name="x2p", bufs=1) as x2p:
            ones_f = tl.tile([128, 1], f32, tag="onesf", name="ones_f", bufs=1)
            nc.vector.memset(ones_f[:], 1.0)
            ones_col = tl.tile([128, 1], f32r, tag="onesc", name="ones_col",
                               bufs=1)
            nc.scalar.copy(ones_col[:], ones_f[:])
            ones_rf = tl.tile([1, 128], f32, tag="onesrf", name="ones_rf",
                              bufs=1)
            nc.vector.memset(ones_rf[:], 1.0)
            ones_row = tl.tile([1, 128], f32r, tag="onesr", name="ones_row",
                               bufs=1)
            nc.scalar.copy(ones_row[:], ones_rf[:])

            x2w = x2p.tile([128, 8, S], f32, name="x2w")
            ssq_f = ps_bcf.tile([1, S], f32, tag="ssqf", name="ssq_f")
            rt_ps = ps_row.tile([E_MLP + E_VE, S], f32, tag="rt", name="rt_ps")
            wo_tiles = []
            for co in range(8):
                wo_t = tl.tile([128, 8, 128], f32r, tag="wo",
                               name=f"wo{co}", bufs=5)
                nc.sync.dma_start(wo_t[:], wo_sc[:, :, bass.ts(co, 128)])
                wo_tiles.append(wo_t)
            sqfs = []
            for co in range(8):
                at_ps = ps_at.tile([128, S], f32, tag="at", name="at_ps")
                for cc in range(8):
                    nc.tensor.matmul(
                        at_ps[:], wo_tiles[co][:, cc, :],
                        yTp[:, cc, :],
                        start=(cc == 0), stop=(cc == 7))
                nc.vector.tensor_add(x2w[:, co, :], at_ps[:], x_t[:, co, :])
                nc.sync.dma_start(x2_out[:, co, :], x2w[:, co, :])
                sqf = tl.tile([128, S], f32r, tag="sqf", name=f"sqf{co}",
                              bufs=8)
                nc.scalar.activation(sqf[:], x2w[:, co, :], AF.Square)
                sqfs.append(sqf)
            for co in range(8):
                nc.tensor.matmul(ssq_f[:], ones_col[:], sqfs[co][:],
                                 start=(co == 0), stop=(co == 7))
                nc.tensor.matmul(rt_ps[:], rw_t[:, co, :], x2w[:, co, :],
                                 start=(co == 0), stop=(co == 7))

            srow = tl.tile([1, S], f32, tag="srow", name="srow", bufs=1)
            rrow = tl.tile([1, S], f32r, tag="rrow", name="rrow", bufs=1)
            bcf_sb = tl.tile([128, S], f32, tag="bcfs", name="bcf_sb", bufs=1)
            xfb = x2p.tile([128, 8, S], f32, name="xfb")
            lg = tl.tile([E_MLP + E_VE, S], f32, tag="lg", name="lg", bufs=1)
            for hf in range(2):
                fsl = bass.ts(hf, SW)
                nc.scalar.activation(srow[0:1, fsl], ssq_f[0:1, fsl],
                                     AF.Sqrt, bias=EPS, scale=1.0 / C)
                with nc.allow_low_precision(reason="f32r rms bcast rows"):
                    nc.vector.reciprocal(rrow[0:1, fsl], srow[0:1, fsl])
                bcf_ps = ps_row.tile([128, SW], f32, tag="bcf", name="bcf_ps")
                nc.tensor.matmul(bcf_ps[:], ones_row[:], rrow[0:1, fsl],
                                 start=True, stop=True)
                nc.vector.tensor_copy(bcf_sb[:, fsl], bcf_ps[:])
                nc.vector.tensor_mul(lg[:, fsl], rt_ps[:, fsl],
                                     bcf_sb[0:E_MLP + E_VE, fsl])
                for co in range(8):
                    eng = nc.vector if co % 2 == 0 else nc.gpsimd
                    eng.tensor_mul(xfb[:, co, fsl], x2w[:, co, fsl],
                                   bcf_sb[:, fsl])
                    nc.sync.dma_start(xfb_out[:, co, fsl], xfb[:, co, fsl])
            nc.sync.dma_start(lg_out[:], lg[:])

    nc.compile()
    return nc


# --------------------------------------------------------------------------
# L2: expert MLP (bf16 up, fp8-DR down, gate pre-folded) + VE sum
# --------------------------------------------------------------------------
def build_2(ncap: int):
    nc = bacc.Bacc("TRN2", target_bir_lowering=False, debug=False,
                   num_devices=NCORES)
    NT = ncap // 512

    xf_hi = nc.dram_tensor("xf_hi", [128, 8, ncap], fp8e4,
                           kind="ExternalInput").ap()
    xf_lo = nc.dram_tensor("xf_lo", [128, 8, ncap], fp8e4,
                           kind="ExternalInput").ap()
    wup = nc.dram_tensor("wup", [128, 8, HID], fp8e4,
                         kind="ExternalInput").ap()
    wdn = nc.dram_tensor("wdn", [128, 16, C], fp8e4,
                         kind="ExternalInput").ap()
    ve0 = nc.dram_tensor("ve0", [128, QT, C], fp8e4, kind="ExternalInput").ap()
    ve1 = nc.dram_tensor("ve1", [128, QT, C], fp8e4, kind="ExternalInput").ap()

    moe_out = nc.dram_tensor("moe_out", [128, 8, ncap], bf16,
                             kind="ExternalOutput").ap()
    ve_out = nc.dram_tensor("ve_out", [128, QT, C], bf16,
                            kind="ExternalOutput").ap()

    with tile.TileContext(nc) as tc, ExitStack() as est:
        wp = est.enter_context(tc.tile_pool(name="wp", bufs=1))
        hp = est.enter_context(tc.tile_pool(name="hp", bufs=2))
        op = est.enter_context(tc.tile_pool(name="op", bufs=3))
        ps_h = est.enter_context(tc.tile_pool(name="ps_h", bufs=3, space="PSUM"))
        ps_o = est.enter_context(tc.tile_pool(name="ps_o", bufs=3, space="PSUM"))

        xh_t = wp.tile([128, 8, ncap], fp8e4, name="xh_t")
        xl_t = wp.tile([128, 8, ncap], fp8e4, name="xl_t")
        up_t = wp.tile([128, 8, HID], fp8e4, name="up_t")
        dn_t = wp.tile([128, 16, C], fp8e4, name="dn_t")
        v0_t = wp.tile([128, QT, C], fp8e4, name="v0_t")
        v1_t = wp.tile([128, QT, C], fp8e4, name="v1_t")
        nc.sync.dma_start(xh_t[:], xf_hi[:])
        for cc in range(4):
            nc.sync.dma_start(up_t[:, 2 * cc:2 * cc + 2, :],
                              wup[:, 2 * cc:2 * cc + 2, :])
        nc.sync.dma_start(xl_t[:], xf_lo[:])
        for cc in range(4):
            nc.sync.dma_start(dn_t[:, 4 * cc:4 * cc + 4, :],
                              wdn[:, 4 * cc:4 * cc + 4, :])
        nc.sync.dma_start(v0_t[:], ve0[:])
        nc.sync.dma_start(v1_t[:], ve1[:])

        for nt in range(NT):
            csl = bass.ts(nt, 512)
            h_hi = hp.tile([128, 16, 512], fp8e4, tag="hh", name=f"hh{nt}")
            h_lo = hp.tile([128, 16, 512], fp8e4, tag="hl", name=f"hl{nt}")
            for hc in range(16):
                h_ps = ps_h.tile([128, 512], f32, tag="hps", name="h_ps")
                for cc in range(4):
                    nc.tensor.matmul(
                        h_ps[:], up_t[:, 2 * cc:2 * cc + 2, bass.ts(hc, 128)],
                        xh_t[:, 2 * cc:2 * cc + 2, csl],
                        start=(cc == 0), stop=False, perf_mode=DR)
                for cc in range(4):
                    nc.tensor.matmul(
                        h_ps[:], up_t[:, 2 * cc:2 * cc + 2, bass.ts(hc, 128)],
                        xl_t[:, 2 * cc:2 * cc + 2, csl],
                        start=False, stop=(cc == 3), perf_mode=DR)
                hr = op.tile([128, 512], f32, tag="hr", name="hr")
                nc.scalar.activation(hr[:], h_ps[:], AF.Relu)
                hsq = op.tile([128, 512], f32, tag="hsq", name="hsq")
                nc.vector.tensor_mul(hsq[:], hr[:], hr[:])
                eng = nc.vector if hc % 2 == 0 else nc.gpsimd
                eng.tensor_copy(h_hi[:, hc, :], hsq[:])
                eng2 = nc.gpsimd if hc % 2 == 0 else nc.vector
                eng2.tensor_sub(h_lo[:, hc, :], hsq[:], h_hi[:, hc, :])
            for co in range(8):
                o_ps = ps_o.tile([128, 512], f32, tag="ops", name="o_ps")
                for hh in range(8):
                    nc.tensor.matmul(
                        o_ps[:], dn_t[:, 2 * hh:2 * hh + 2, bass.ts(co, 128)],
                        h_hi[:, 2 * hh:2 * hh + 2, :],
                        start=(hh == 0), stop=False, perf_mode=DR)
                for hh in range(8):
                    nc.tensor.matmul(
                        o_ps[:], dn_t[:, 2 * hh:2 * hh + 2, bass.ts(co, 128)],
                        h_lo[:, 2 * hh:2 * hh + 2, :],
                        start=False, stop=(hh == 7), perf_mode=DR)
                ot = op.tile([128, 512], bf16, tag="ot", name="ot")
                nc.scalar.copy(ot[:], o_ps[:])
                nc.sync.dma_start(moe_out[:, co, csl], ot[:])

        vo = op.tile([128, QT, C], bf16, tag="vo", name="vo", bufs=1)
        for t in range(QT):
            eng = nc.vector if t % 2 == 0 else nc.gpsimd
            eng.tensor_add(vo[:, t, :], v0_t[:, t, :], v1_t[:, t, :])
        nc.sync.dma_start(ve_out[:], vo[:])

    nc.compile()
    return nc


# --------------------------------------------------------------------------
# Host orchestration
# --------------------------------------------------------------------------
def _qtiles(ci):
    # strip A = 256-tile ci (128-tiles 2ci, 2ci+1), strip B = 256-tile 7-ci
    sa, sb = ci, 7 - ci
    return [2 * sa, 2 * sa + 1, 2 * sb, 2 * sb + 1]


def _slot_map(st, n_s, window):
    """slot -> (key 128-tile, bias).  Diagonal tile (2st+1) at slot n_s-1,
    sub-diagonal (2st) at n_s-2, other causally-alive tiles from slot 0."""
    alive = [kt for kt in range(2 * st)
             if window >= T or 128 * (2 * st + 1 - kt) - 127 <= window]
    m = {}
    for si, kt in enumerate(alive):
        m[si] = (kt, 0.0)
    m[n_s - 2] = (2 * st, 0.0)
    m[n_s - 1] = (2 * st + 1, 0.0)
    return m


def _route(logits, router_bias):
    sig = (1.0 / (1.0 + np.exp(-logits.astype(np.float32)))).astype(np.float32)
    sel = sig + router_bias[None, :].astype(np.float32)
    idx = np.argsort(-sel, axis=1, kind="stable")[:, :TOPK]
    tw = np.take_along_axis(sig, idx, axis=1)
    tw = tw / tw.sum(axis=1, keepdims=True)
    N = logits.shape[0]
    sparse_w = np.zeros((N, E_MLP + E_VE), np.float32)
    np.put_along_axis(sparse_w, idx, tw, axis=1)
    return sparse_w


def kernel(**inputs):
    x = np.asarray(inputs["x"], np.float32)
    token_ids = np.asarray(inputs["token_ids"])
    cos = np.asarray(inputs["cos"], np.float32)
    sin = np.asarray(inputs["sin"], np.float32)
    window = int(np.asarray(inputs["window_size"]))
    wq, wk, wv, wo = (np.asarray(inputs[k], np.float32)
                      for k in ("wq", "wk", "wv", "wo"))
    w_up = np.asarray(inputs["w_up"], np.float32)
    w_down = np.asarray(inputs["w_down"], np.float32)
    router_w = np.asarray(inputs["router_w"], np.float32)
    router_bias = np.asarray(inputs["router_bias"], np.float32)
    ve_tables = np.asarray(inputs["ve_tables"], np.float32)

    cosf = cos[0, :, 0, :]      # (T, 32)
    sinf = sin[0, :, 0, :]

    # ---------------- L1a ----------------
    if "1a" not in _prog_cache:
        _prog_cache["1a"] = build_1a()
    nc1a = _prog_cache["1a"]

    def mov_w(w):  # (M, C) -> [128, 8, M] with [p, cc, m] = w[m, 128cc+p]
        M = w.shape[0]
        return np.ascontiguousarray(
            w.T.reshape(8, 128, M).transpose(1, 0, 2)).astype(np.float32)

    wq_t, wk_t, wv_t = mov_w(wq), mov_w(wk), mov_w(wv)
    toks_all, maps1a = [], []
    for c in range(NCORES):
        b, ci = c // 4, c % 4
        qts = _qtiles(ci)
        toks = np.concatenate([np.arange(qt * 128, qt * 128 + 128)
                               for qt in qts])
        toks_all.append(toks)
        xs = x[b, toks, :]                      # (512, 1024)
        x_fm = np.ascontiguousarray(
            xs.T.reshape(8, 128, S).transpose(1, 0, 2)).astype(np.float32)
        x_tm = np.ascontiguousarray(
            xs.reshape(QT, 128, C).transpose(1, 0, 2)).astype(np.float32)
        cos_tm = np.ascontiguousarray(
            cosf[toks].reshape(QT, 128, 32).transpose(1, 0, 2)
        ).astype(np.float32)
        sin_tm = np.ascontiguousarray(
            sinf[toks].reshape(QT, 128, 32).transpose(1, 0, 2)
        ).astype(np.float32)
        maps1a.append(dict(x_fm=x_fm, x_tm=x_tm, wq_t=wq_t, wk_t=wk_t,
                           wv_t=wv_t, cos_tm=cos_tm, sin_tm=sin_tm))
    res1a = run_bass_kernel_spmd(nc1a, maps1a, list(range(NCORES))).results

    kn = np.zeros((B, T, NKV * HD), np.float32)
    vn = np.zeros((B, T, NKV * HD), np.float32)
    qn = []
    for c in range(NCORES):
        b = c // 4
        toks = toks_all[c]
        kc = res1a[c]["k_out"].astype(np.float32)   # [128, QT, 512]
        vc = res1a[c]["v_out"].astype(np.float32)
        kn[b, toks.reshape(QT, 128)] = kc.transpose(1, 0, 2)
        vn[b, toks.reshape(QT, 128)] = vc.transpose(1, 0, 2)
        qn.append(res1a[c]["q_out"].astype(np.float32))  # [128, QT, 1024]

    # ---------------- L1b ----------------
    masked = window < T
    key1b = ("1b", masked)
    if key1b not in _prog_cache:
        _prog_cache[key1b] = build_1b(masked)
    nc1b = _prog_cache[key1b]

    wo_sc = np.ascontiguousarray(
        wo.T.reshape(8, 128, C).transpose(1, 0, 2)).astype(np.float32)
    rw_sb = np.ascontiguousarray(
        router_w.T.reshape(8, 128, E_MLP + E_VE).transpose(1, 0, 2)
    ).astype(np.float32)

    maps1b = []
    for c in range(NCORES):
        b, ci = c // 4, c % 4
        strips = [ci, 7 - ci]          # 256-token strip indices
        toks = toks_all[c]
        q_sc = np.zeros((65, NH, NSTRIP, SW), np.float32)
        qc = qn[c]                     # [128, QT, 1024] token-major
        qtok = qc.transpose(1, 0, 2).reshape(S, NH, HD)   # (512, NH, 64)
        q_sc[0:64] = qtok.reshape(NSTRIP, SW, NH, HD).transpose(3, 2, 0, 1)
        q_sc[64] = 1.0
        kt_sc = np.zeros((65, NKV, NSLOT_TOT, 128), np.float32)
        kt_sc[64] = DEAD_BIAS
        v_sc = np.zeros((128, NKV, NSLOT_TOT, 65), np.float32)
        v_sc[:, :, :, 64] = 1.0
        kb = kn[b].reshape(16, 128, NKV, HD)   # [ktile, n, kv, d]
        vb = vn[b].reshape(16, 128, NKV, HD)
        for strip in range(NSTRIP):
            st = strips[strip]
            n_s = NPOS[strip]
            smap = _slot_map(st, n_s, window)
            for s, (kt, bias) in smap.items():
                po = POS_BASE[strip] + s
                kt_sc[0:64, :, po, :] = kb[kt].transpose(2, 1, 0)
                kt_sc[64, :, po, :] = bias
                v_sc[:, :, po, 0:64] = vb[kt]
        xs = x[b, toks, :]
        x_fm32 = np.ascontiguousarray(
            xs.T.reshape(8, 128, S).transpose(1, 0, 2)).astype(np.float32)
        m = dict(q_sc=q_sc, kt_sc=np.ascontiguousarray(kt_sc),
                 v_sc=np.ascontiguousarray(v_sc), x_fm32=x_fm32,
                 wo_sc=wo_sc, rw_sb=rw_sb)
        if masked:
            wm = np.zeros((128, NSLOT_TOT * SW), np.float32)
            for strip in range(NSTRIP):
                st = strips[strip]
                n_s = NPOS[strip]
                smap = _slot_map(st, n_s, window)
                for s, (kt, bias) in smap.items():
                    po = POS_BASE[strip] + s
                    qg = 2 * st * 128 + np.arange(SW)[None, :]
                    kg = kt * 128 + np.arange(128)[:, None]
                    ok = (kg <= qg) & (qg - kg <= window)
                    wm[:, po * SW:(po + 1) * SW] = ok
            m["wmask"] = wm.astype(np.float32)
        maps1b.append(m)
    res1b = run_bass_kernel_spmd(nc1b, maps1b, list(range(NCORES))).results

    # ---------------- routing ----------------
    N = B * T
    logits = np.zeros((N, E_MLP + E_VE), np.float32)
    x2 = np.zeros((N, C), np.float32)
    xfb = np.zeros((N, C), np.float32)
    for c in range(NCORES):
        b = c // 4
        toks = toks_all[c] + b * T
        logits[toks] = res1b[c]["lg_out"].T
        x2c = res1b[c]["x2_out"]                     # [128 p, 8 cc, 512 n]
        x2[toks] = x2c.transpose(2, 1, 0).reshape(S, C)
        xfc = res1b[c]["xfb_out"].astype(np.float32)
        xfb[toks] = xfc.transpose(2, 1, 0).reshape(S, C)

    sparse_w = _route(logits, router_bias)

    idx_list = [np.nonzero(sparse_w[:, e])[0] for e in range(E_MLP)]
    n_list = [len(ix) for ix in idx_list]
    ncap = NCAP0
    while ncap < max(n_list):
        ncap += 512

    key2 = ("2", ncap)
    if key2 not in _prog_cache:
        _prog_cache[key2] = build_2(ncap)
    nc2 = _prog_cache[key2]

    tok_flat = token_ids.reshape(-1)
    maps2 = []
    for c in range(NCORES):
        e = c
        ix = idx_list[e]
        g = np.sqrt(sparse_w[ix, e]).astype(np.float32)
        xg = np.zeros((C, ncap), np.float32)
        xg[:, :len(ix)] = xfb[ix].T * g[None, :]
        xfg = np.ascontiguousarray(
            xg.reshape(8, 128, ncap).transpose(1, 0, 2))
        xf_hi = xfg.astype(E4)
        xf_lo = (xfg - xf_hi.astype(np.float32)).astype(E4)
        wup_m = np.ascontiguousarray(
            w_up[e].T.reshape(8, 128, HID).transpose(1, 0, 2)).astype(E4)
        wdn_dr = np.ascontiguousarray(
            w_down[e].T.reshape(16, 128, C).transpose(1, 0, 2)).astype(E4)
        b = c // 4
        toks = toks_all[c] + b * T
        tids = tok_flat[toks]
        ve0 = (64.0 * sparse_w[toks, E_MLP, None]
               * ve_tables[0][tids]).reshape(QT, 128, C).transpose(1, 0, 2)
        ve1 = (64.0 * sparse_w[toks, E_MLP + 1, None]
               * ve_tables[1][tids]).reshape(QT, 128, C).transpose(1, 0, 2)
        maps2.append(dict(xf_hi=xf_hi, xf_lo=xf_lo, wup=wup_m, wdn=wdn_dr,
                          ve0=np.ascontiguousarray(ve0).astype(E4),
                          ve1=np.ascontiguousarray(ve1).astype(E4)))
    res2 = run_bass_kernel_spmd(nc2, maps2, list(range(NCORES))).results

    out = x2.copy()
    for c in range(NCORES):
        b = c // 4
        toks = toks_all[c] + b * T
        veo = res2[c]["ve_out"].astype(np.float32)  # [128, QT, C]
        out[toks] += veo.transpose(1, 0, 2).reshape(S, C) / 64.0
    for e in range(E_MLP):
        n_e = n_list[e]
        if n_e:
            moe = res2[e]["moe_out"].astype(np.float32)  # [128, 8, ncap]
            out[idx_list[e]] += moe[:, :, :n_e].transpose(
                2, 1, 0).reshape(n_e, C)
    return out.reshape(B, T, C).astype(np.float32)
